# revision 1
# baseline (speedup 1.0000x reference)
"""Trainium2 Bass kernel for nn_Attention_49641232007688 (sparse_attention).

Data-parallel over batch B=8 across 8 NeuronCores (one batch element per
core). Per core, everything runs in fp16 on-device (fp32 PSUM accumulation):

  1. depthwise 3x3 convs (BN-folded) on DVE via per-partition-scalar MACs
     over a host-padded [58,58] buffer (a host-shifted copy keeps every
     MAC 4B-aligned for the DVE 2x perf mode)
  2. 1x1 convs as PE matmuls producing q,v in channel-major layout [hd, n]
     (A) and k,v in position-major layout [n, hd] (B, fused chunk-wise into
     the ktv accumulation); k in layout A is consumed chunk-wise straight
     from PSUM by the diag pipeline
  3. diag = per-head sum(q*k) via a block-ones PE matmul; the global scalar
     m0 = sum(diag) via DVE/GPSIMD reduce + a 1-element DRAM AllReduce over
     the 8 cores
  4. ktv = k^T v per head (PE, contraction over n via layout B);
     out = q@ktv + (m0-diag)*v accumulated in PSUM (negated-identity matmul
     folds the elementwise term into the same PSUM accumulation group)
  5. final 1x1 conv (PE) + bias via ACT eviction, fp32 output streamed out
     per chunk.

The module is built once and cached; kernel() accepts full inputs and
returns the full output.
"""

import numpy as np

HEADS = 8
DIM = 384
DIM_HEAD = 64
INNER = 512
B = 8
H = W = 56
N = H * W            # 3136
EPS = 1e-5
SCALE = DIM_HEAD ** -0.5
PC = 58              # padded cols/rows
XL = PC * PC + 4     # padded x flat length (+4 OOB slack for shifted reads)
YL = H * PC          # conv output flat length (56 rows x 58 cols)
NCH = 7              # n-chunks of 448 (8 rows) for 448-wide matmuls
CHW = 448
BCH = 28             # n-chunks of 112 (2 rows) for layout-B stationaries
CP = 3               # channel partition tiles (384 = 3*128)
HP = 4               # head-dim partition tiles (512 = 4*128)

_CACHE = {}


def _f16(a):
    return np.ascontiguousarray(a, dtype=np.float16)


def _build(reps: int = 1, loop_n=None, no_cc=False):
    import concourse.bacc as bacc
    import concourse.mybir as mybir
    import concourse.tile as tile

    F16 = mybir.dt.float16
    F32 = mybir.dt.float32
    ALU = mybir.AluOpType
    ACTF = mybir.ActivationFunctionType

    nc = bacc.Bacc(None, num_devices=8)

    # ---- DRAM I/O ----
    xp_d = nc.dram_tensor("xp", [DIM, XL], F16, kind="ExternalInput")
    xp1_d = nc.dram_tensor("xp1", [DIM, XL], F16, kind="ExternalInput")
    tq_d = nc.dram_tensor("tq", [DIM, 9], F32, kind="ExternalInput")
    bq_d = nc.dram_tensor("bq", [DIM, 1], F32, kind="ExternalInput")
    tk_d = nc.dram_tensor("tk", [DIM, 9], F32, kind="ExternalInput")
    bk_d = nc.dram_tensor("bk", [DIM, 1], F32, kind="ExternalInput")
    wqT_d = nc.dram_tensor("wqT", [DIM, INNER], F16, kind="ExternalInput")
    wkvT_d = nc.dram_tensor("wkvT", [DIM, 2 * INNER], F16, kind="ExternalInput")
    woT_d = nc.dram_tensor("woT", [INNER, DIM], F16, kind="ExternalInput")
    bo_d = nc.dram_tensor("bo", [DIM, 1], F32, kind="ExternalInput")
    hsel_d = nc.dram_tensor("hsel", [INNER, HEADS], F16, kind="ExternalInput")
    hselT_d = nc.dram_tensor("hselT", [HEADS, INNER], F16, kind="ExternalInput")
    ones8_d = nc.dram_tensor("ones8", [HEADS, 1], F16, kind="ExternalInput")
    negI_d = nc.dram_tensor("negI", [128, 128], F16, kind="ExternalInput")
    posI_d = nc.dram_tensor("posI", [128, 128], F16, kind="ExternalInput")
    out_d = nc.dram_tensor("out", [DIM, N], F32, kind="ExternalOutput")

    def ch_cols(t, ch):
        # columns of an [*, N]-wide sbuf tile for n-chunk ch
        return t[:, ch * CHW:(ch + 1) * CHW]

    with tile.TileContext(nc) as tc:
        with (
            tc.tile_pool(name="wsb", bufs=1) as wsb,
            tc.tile_pool(name="xsb", bufs=1) as xsb,
            tc.tile_pool(name="ysb", bufs=1) as ysb,
            tc.tile_pool(name="big", bufs=1) as big,
            tc.tile_pool(name="small", bufs=2) as small,
            tc.tile_pool(name="ev", bufs=3) as ev,
            tc.tile_pool(name="mm_ps", bufs=3, space="PSUM") as mm_ps,
            tc.tile_pool(name="kv_ps", bufs=2, space="PSUM") as kv_ps,
            tc.tile_pool(name="ktv_ps", bufs=1, space="PSUM") as ktv_ps,
            tc.tile_pool(name="dps", bufs=1, space="PSUM") as dps_pool,
            tc.tile_pool(name="dram", bufs=1, space="DRAM") as dram,
        ):
            def emit_body():
                # ---- load padded x (first: the convs gate everything) ----
                x0_t = [xsb.tile([128, XL], F16, tag=f"x0{p}", name=f"x0{p}") for p in range(CP)]
                x1_t = [xsb.tile([128, XL], F16, tag=f"x1{p}", name=f"x1{p}") for p in range(CP)]
                G0E = 26 * PC
                G1S = 24 * PC
                for p in range(CP):
                    cs = slice(p * 128, (p + 1) * 128)
                    nc.sync.dma_start(out=x0_t[p][:, 0:G0E], in_=xp_d[cs, 0:G0E])
                    nc.sync.dma_start(out=x1_t[p][:, 0:G0E], in_=xp1_d[cs, 0:G0E])
                for p in range(CP):
                    cs = slice(p * 128, (p + 1) * 128)
                    nc.sync.dma_start(out=x0_t[p][:, G1S:XL], in_=xp_d[cs, G1S:XL])
                    nc.sync.dma_start(out=x1_t[p][:, G1S:XL], in_=xp1_d[cs, G1S:XL])

                # ---- load weights ----
                wq_t = [wsb.tile([128, INNER], F16, tag=f"wq{p}", name=f"wq{p}") for p in range(CP)]
                wkv_t = [wsb.tile([128, 2 * INNER], F16, tag=f"wkv{p}", name=f"wkv{p}") for p in range(CP)]
                wo_t = [wsb.tile([128, DIM], F16, tag=f"wo{p}", name=f"wo{p}") for p in range(HP)]
                tq_t = [wsb.tile([128, 9], F32, tag=f"tq{p}", name=f"tq{p}") for p in range(CP)]
                bq_t = [wsb.tile([128, 1], F32, tag=f"bq{p}", name=f"bq{p}") for p in range(CP)]
                tk_t = [wsb.tile([128, 9], F32, tag=f"tk{p}", name=f"tk{p}") for p in range(CP)]
                bk_t = [wsb.tile([128, 1], F32, tag=f"bk{p}", name=f"bk{p}") for p in range(CP)]
                bo_t = [wsb.tile([128, 1], F32, tag=f"bo{p}", name=f"bo{p}") for p in range(CP)]
                hsel_t = [wsb.tile([128, HEADS], F16, tag=f"hs{p}", name=f"hs{p}") for p in range(HP)]
                hselT_t = wsb.tile([HEADS, INNER], F16, tag="hsT", name="hsT")
                ones8_t = wsb.tile([HEADS, 1], F16, tag="ones8", name="ones8")
                negI_t = wsb.tile([128, 128], F16, tag="negI", name="negI")
                posI_t = wsb.tile([128, 128], F16, tag="posI", name="posI")
                for p in range(CP):
                    cs = slice(p * 128, (p + 1) * 128)
                    nc.sync.dma_start(out=wq_t[p], in_=wqT_d[cs, :])
                    nc.sync.dma_start(out=wkv_t[p], in_=wkvT_d[cs, :])
                    nc.sync.dma_start(out=tq_t[p], in_=tq_d[cs, :])
                    nc.sync.dma_start(out=bq_t[p], in_=bq_d[cs, :])
                    nc.sync.dma_start(out=tk_t[p], in_=tk_d[cs, :])
                    nc.sync.dma_start(out=bk_t[p], in_=bk_d[cs, :])
                    nc.sync.dma_start(out=bo_t[p], in_=bo_d[cs, :])
                for p in range(HP):
                    cs = slice(p * 128, (p + 1) * 128)
                    nc.sync.dma_start(out=wo_t[p], in_=woT_d[cs, :])
                    nc.sync.dma_start(out=hsel_t[p], in_=hsel_d[cs, :])
                nc.sync.dma_start(out=hselT_t, in_=hselT_d[:, :])
                nc.sync.dma_start(out=ones8_t, in_=ones8_d[:, :])
                nc.sync.dma_start(out=negI_t, in_=negI_d[:, :])
                nc.sync.dma_start(out=posI_t, in_=posI_d[:, :])

                # ---- depthwise convs (DVE) ----
                # scalar_tensor_tensor has no fast DVE uop (1x); instead each
                # tap is tensor_scalar mul (4x mode, contiguous padded stream
                # into a temp) + tensor_tensor add (2x mode, strided temp view
                # onto the compact accumulator): 2.57us vs 3.33us per tap.
                # Emitted in two row-groups (group-major across c-tiles) so
                # downstream matmuls on early n-chunks start at half-time.
                RG = ((0, 24), (24, H))

                def conv_group(ys, taps, bias, r0, r1):
                    rows = r1 - r0
                    ve = nc.vector
                    if True:
                        for p in range(CP):
                            y = ys[p]
                            x0v = x0_t[p][:, 0:PC * PC].rearrange("p (a b) -> p a b", b=PC)
                            x1v = x1_t[p][:, 0:PC * PC].rearrange("p (a b) -> p a b", b=PC)
                            first = True
                            for dy in range(3):
                                for dx in range(3):
                                    i = dy * 3 + dx
                                    if dx == 1:
                                        src = x1v[:, r0 + dy:r1 + dy, 0:W]
                                    else:
                                        src = x0v[:, r0 + dy:r1 + dy, dx:dx + W]
                                    if first:
                                        ve.tensor_scalar(
                                            out=y[:, r0:r1, :], in0=src,
                                            scalar1=taps[p][:, i:i + 1],
                                            scalar2=bias[p],
                                            op0=ALU.mult, op1=ALU.add)
                                        first = False
                                    else:
                                        off = (r0 + dy) * PC + dx
                                        glen = rows * PC
                                        t = ysb.tile([128, glen], F16, tag="tconv",
                                                     name="tconv", bufs=2,
                                                     padded_shape=[128, YL // 2 + PC * 4])
                                        if off % 2 == 0:
                                            tsrc = x0_t[p][:, off:off + glen]
                                        else:
                                            tsrc = x1_t[p][:, off - 1:off - 1 + glen]
                                        ve.tensor_scalar(
                                            out=t, in0=tsrc,
                                            scalar1=taps[p][:, i:i + 1],
                                            scalar2=None, op0=ALU.mult)
                                        tv = t.rearrange("p (a b) -> p a b", b=PC)
                                        ve.tensor_tensor(
                                            out=y[:, r0:r1, :], in0=tv[:, 0:rows, 0:W],
                                            in1=y[:, r0:r1, :], op=ALU.add)
                yq_t = [ysb.tile([128, H, W], F16, tag=f"yq{p}", name=f"yq{p}")
                        for p in range(CP)]
                ykv_t = [ysb.tile([128, H, W], F16, tag=f"ykv{p}", name=f"ykv{p}")
                         for p in range(CP)]
                for (r0, r1) in RG:
                    conv_group(ykv_t, tk_t, bk_t, r0, r1)
                for (r0, r1) in RG:
                    conv_group(yq_t, tq_t, bq_t, r0, r1)
                y_kv = [y.rearrange("p a b -> p (a b)") for y in ykv_t]
                y_q = [y.rearrange("p a b -> p (a b)") for y in yq_t]

                def y_chunk(y, ch):
                    return y[:, ch * CHW:(ch + 1) * CHW]

                # ---- layout-A projections q, v: [512, 3136] fp16 ----
                def proj_A(ys, wts, col0, name):
                    dst = [big.tile([128, N], F16, tag=f"{name}{m}", name=f"{name}{m}") for m in range(HP)]
                    for m in range(HP):
                        for ch in range(NCH):
                            ps = mm_ps.tile([128, CHW], F32, tag="mm", name="mm")
                            for p in range(CP):
                                nc.tensor.matmul(
                                    out=ps[:, :],
                                    lhsT=wts[p][:, col0 + m * 128: col0 + (m + 1) * 128],
                                    rhs=y_chunk(ys[p], ch),
                                    start=(p == 0), stop=(p == CP - 1))
                            nc.scalar.copy(out=ch_cols(dst[m], ch), in_=ps[:, :])
                    return dst

                vA = proj_A(y_kv, wkv_t, INNER, "vA")
                qA = proj_A(y_q, wq_t, 0, "qA")

                # ---- layout-B k,v chunks fused into ktv accumulation ----
                # Per n-chunk: head-pair block matmuls (complete PSUM groups)
                # -> ACT evicts to fp16 -> PE identity-matmul accumulates into
                # a long-lived PSUM bank (no DVE involvement).
                ktv_acc = ktv_ps.tile([128, HP, 128], F32, tag="ktva", name="ktva")
                for bc in range(BCH):
                    kvch = []
                    for half in range(2):
                        ps = kv_ps.tile([112, INNER], F32, tag="kv", name="kv")
                        for p in range(CP):
                            nc.tensor.matmul(
                                out=ps[:, :],
                                lhsT=y_kv[p][:, bc * 112:(bc + 1) * 112],
                                rhs=wkv_t[p][:, half * INNER:(half + 1) * INNER],
                                start=(p == 0), stop=(p == CP - 1))
                        sb = ev.tile([112, INNER], F16, tag=f"kv16_{half}", name=f"kv16_{half}")
                        nc.scalar.copy(out=sb, in_=ps[:, :])
                        kvch.append(sb)
                    kt_ps = ktv_ps.tile([128, HP, 128], F32, tag="ktvp", name="ktvp")
                    for mp in range(HP):
                        ms = slice(mp * 128, (mp + 1) * 128)
                        nc.tensor.matmul(out=kt_ps[:, mp, :],
                                         lhsT=kvch[0][:, ms],
                                         rhs=kvch[1][:, ms],
                                         start=True, stop=True)
                    kt16 = ev.tile([128, HP * 128], F16, tag="kt16", name="kt16", bufs=2)
                    nc.scalar.copy(out=kt16, in_=kt_ps.rearrange("p a b -> p (a b)"))
                    nc.tensor.matmul(out=ktv_acc.rearrange("p a b -> p (a b)"),
                                     lhsT=posI_t, rhs=kt16,
                                     start=(bc == 0), stop=(bc == BCH - 1))
                # block-diagonal [ktv_2m, ktv_2m+1] per m-tile: term2 for a
                # whole m-tile is then ONE K=128 matmul sharing base
                # partition 0 with qA[m].
                ktv_bd = small.tile([128, HP, 128], F16, tag="ktvbd", name="ktvbd")
                nc.vector.memset(ktv_bd, 0.0)
                nc.scalar.copy(out=ktv_bd[0:DIM_HEAD, :, 0:DIM_HEAD],
                               in_=ktv_acc[0:DIM_HEAD, :, 0:DIM_HEAD])
                nc.scalar.copy(out=ktv_bd[DIM_HEAD:128, :, DIM_HEAD:128],
                               in_=ktv_acc[DIM_HEAD:128, :, DIM_HEAD:128])

                # ---- diag + m0 (k in layout A consumed straight from PSUM) ----
                diag16 = big.tile([HEADS, N], F16, tag="diag16", name="diag16")
                for ch in range(NCH):
                    dps = dps_pool.tile([HEADS, CHW], F32, tag="dps", name="dps")
                    for m in range(HP):
                        kps = mm_ps.tile([128, CHW], F32, tag="mm", name="mm")
                        for p in range(CP):
                            nc.tensor.matmul(
                                out=kps[:, :],
                                lhsT=wkv_t[p][:, m * 128:(m + 1) * 128],
                                rhs=y_chunk(y_kv[p], ch),
                                start=(p == 0), stop=(p == CP - 1))
                        k16 = ev.tile([128, CHW], F16, tag="k16", name="k16")
                        nc.scalar.copy(out=k16, in_=kps[:, :])
                        qk = ev.tile([128, CHW], F16, tag="qk", name="qk")
                        nc.vector.tensor_tensor(out=qk, in0=k16,
                                                in1=ch_cols(qA[m], ch), op=ALU.mult)
                        nc.tensor.matmul(out=dps[:, :], lhsT=hsel_t[m], rhs=qk,
                                         start=(m == 0), stop=(m == HP - 1))
                    nc.scalar.copy(out=ch_cols(diag16, ch), in_=dps[:, :])

                # wov = Wo @ v, m0-independent; the final matmul adds m0*wov
                # as one extra accumulation, so the whole oa pipeline closes
                # before the collective result arrives. Emitted chunk-wise
                # inside the main loop below (lower priority than diag).
                wov = [big.tile([128, N], F16, tag=f"wov{p}", name=f"wov{p}") for p in range(CP)]

                dsum = small.tile([HEADS, 1], F32, tag="dsum", name="dsum")
                nc.vector.tensor_reduce(out=dsum, in_=diag16,
                                        axis=mybir.AxisListType.X, op=ALU.add)
                dsum16 = small.tile([HEADS, 1], F16, tag="dsum16", name="dsum16")
                nc.vector.tensor_copy(out=dsum16, in_=dsum)
                m0_ps = dps_pool.tile([1, 1], F32, tag="dps", name="m0ps")
                nc.tensor.matmul(out=m0_ps[:, :], lhsT=ones8_t, rhs=dsum16,
                                 start=True, stop=True)
                m0s = small.tile([1, 1], F32, tag="m0s", name="m0s")
                nc.scalar.copy(out=m0s, in_=m0_ps[:, :])
                cc = dram.tile([1, 1], F32, tag="cc", name="cc")
                nc.gpsimd.dma_start(out=cc[:, :], in_=m0s)
                if not no_cc:
                    nc.gpsimd.collective_compute(
                        "AllReduce", ALU.add, replica_groups=[list(range(8))],
                        ins=[cc[:, :].opt()], outs=[cc[:, :].opt()])
                m0b = small.tile([128, 1], F32, tag="m0b", name="m0b")
                nc.gpsimd.dma_start(out=m0b, in_=cc[:, :].partition_broadcast(128))
                # m0I = m0 * I -- folds the m0*v term into the oa PSUM group,
                # so everything else in the back half is m0-independent and
                # overlaps the front.
                m0I = small.tile([128, 128], F16, tag="m0I", name="m0I")
                nc.vector.tensor_scalar(out=m0I, in0=posI_t, scalar1=m0b,
                                        scalar2=None, op0=ALU.mult)

                # ---- out_attn + final matmul, streamed per chunk ----
                for ch in range(NCH):
                    for ot in range(CP):
                        ps = mm_ps.tile([128, CHW], F32, tag="mm", name="mm")
                        for kt in range(HP):
                            nc.tensor.matmul(
                                out=ps[:, :],
                                lhsT=wo_t[kt][:, ot * 128:(ot + 1) * 128],
                                rhs=ch_cols(vA[kt], ch),
                                start=(kt == 0), stop=(kt == HP - 1))
                        nc.scalar.copy(out=ch_cols(wov[ot], ch), in_=ps[:, :])
                    oach = []
                    for m in range(HP):
                        sb_ps = kv_ps.tile([128, CHW], F32, tag="kv", name="sbps")
                        nc.tensor.matmul(out=sb_ps[:, :],
                                         lhsT=hselT_t[:, m * 128:(m + 1) * 128],
                                         rhs=ch_cols(diag16, ch),
                                         start=True, stop=True)
                        sb16 = ev.tile([128, CHW], F16, tag="sb16", name="sb16")
                        nc.scalar.copy(out=sb16, in_=sb_ps[:, :])
                        ew = ev.tile([128, CHW], F16, tag="ew", name="ew")
                        nc.vector.tensor_tensor(out=ew, in0=sb16,
                                                in1=ch_cols(vA[m], ch), op=ALU.mult)
                        oa_ps = mm_ps.tile([128, CHW], F32, tag="mm", name="mm")
                        nc.tensor.matmul(out=oa_ps[:, :], lhsT=ktv_bd[:, m, :],
                                         rhs=ch_cols(qA[m], ch),
                                         start=True, stop=False)
                        nc.tensor.matmul(out=oa_ps[:, :], lhsT=negI_t, rhs=ew,
                                         start=False, stop=True)
                        oa = ev.tile([128, CHW], F16, tag=f"oa{m}", name=f"oa{m}")
                        if (ch + m) % 2 == 0:
                            nc.scalar.copy(out=oa, in_=oa_ps[:, :])
                        else:
                            nc.vector.tensor_copy(out=oa, in_=oa_ps[:, :])
                        oach.append(oa)
                    for ot in range(CP):
                        ps = dps_pool.tile([128, CHW], F32, tag="dps", name="fps")
                        for kt in range(HP):
                            nc.tensor.matmul(
                                out=ps[:, :],
                                lhsT=wo_t[kt][:, ot * 128:(ot + 1) * 128],
                                rhs=oach[kt],
                                start=(kt == 0), stop=False)
                        nc.tensor.matmul(out=ps[:, :], lhsT=m0I,
                                         rhs=ch_cols(wov[ot], ch),
                                         start=False, stop=True)
                        of = ev.tile([128, CHW], F32, tag="of", name="of", bufs=2)
                        nc.scalar.activation(out=of, in_=ps[:, :],
                                             func=ACTF.Identity,
                                             bias=bo_t[ot], scale=1.0)
                        nc.sync.dma_start(
                            out=out_d[ot * 128:(ot + 1) * 128,
                                      ch * CHW:(ch + 1) * CHW],
                            in_=of)
            if loop_n is None:
                for _ in range(reps):
                    emit_body()
            else:
                with tc.For_i(0, loop_n, 1):
                    emit_body()
    nc.finalize()
    return nc


def _get_nc(reps: int = 1, loop_n=None, no_cc=False):
    key = (reps, loop_n, no_cc)
    if key not in _CACHE:
        _CACHE[key] = _build(reps, loop_n, no_cc)
    return _CACHE[key]


def prepare_in_maps(inputs):
    """Host-side preprocessing: fold BN, pad/shift x, transpose weights."""
    x = np.asarray(inputs["x"], np.float32)

    def fold(dw, g, b, m, v):
        inv = np.asarray(g, np.float32) / np.sqrt(np.asarray(v, np.float32) + EPS)
        taps = np.asarray(dw, np.float32)[:, 0].reshape(DIM, 9) * inv[:, None]
        bias = np.asarray(b, np.float32) - np.asarray(m, np.float32) * inv
        return (np.ascontiguousarray(taps, np.float32),
                np.ascontiguousarray(bias[:, None], np.float32))

    tq, bq = fold(inputs["wq_dw"], inputs["wq_bn_g"], inputs["wq_bn_b"],
                  inputs["wq_bn_m"], inputs["wq_bn_v"])
    tk, bk = fold(inputs["wkv_dw"], inputs["wkv_bn_g"], inputs["wkv_bn_b"],
                  inputs["wkv_bn_m"], inputs["wkv_bn_v"])
    wqT = _f16((SCALE * np.asarray(inputs["wq_pw"], np.float32)).T)
    wkvT = _f16(np.asarray(inputs["wkv_pw"], np.float32).T)
    woT = _f16(np.asarray(inputs["wo"], np.float32).T)
    bo = np.ascontiguousarray(np.asarray(inputs["bo"], np.float32)[:, None])
    hsel = _f16(np.repeat(np.eye(HEADS, dtype=np.float32), DIM_HEAD, axis=0))
    hselT = _f16(hsel.T)
    negI = _f16(-np.eye(128, dtype=np.float32))
    posI = _f16(np.eye(128, dtype=np.float32))
    ones8 = _f16(np.ones((HEADS, 1), np.float32))

    xpad = np.zeros((B, DIM, PC, PC), np.float16)
    xpad[:, :, 1:1 + H, 1:1 + W] = x.astype(np.float16)
    xflat = np.zeros((B, DIM, XL), np.float16)
    xflat[:, :, :PC * PC] = xpad.reshape(B, DIM, PC * PC)
    xsh = np.zeros_like(xflat)
    xsh[:, :, :XL - 1] = xflat[:, :, 1:]

    shared = dict(tq=tq, bq=bq, tk=tk, bk=bk, wqT=wqT, wkvT=wkvT, woT=woT,
                  bo=bo, hsel=hsel, hselT=hselT, negI=negI, posI=posI, ones8=ones8)
    return [dict(shared, xp=np.ascontiguousarray(xflat[b]),
                 xp1=np.ascontiguousarray(xsh[b])) for b in range(B)]


def kernel(**inputs) -> np.ndarray:
    from concourse.bass_utils import run_bass_kernel_spmd
    in_maps = prepare_in_maps(inputs)
    nc = _get_nc(1)
    res = run_bass_kernel_spmd(nc, in_maps, list(range(8)))
    out = np.stack([res.results[b]["out"] for b in range(B)])
    return np.ascontiguousarray(out.reshape(B, DIM, H, W).astype(np.float32))



# revision 10
# speedup vs baseline: 6.6301x; 6.6301x over previous
"""Trainium2 Bass kernel for nn_Attention_49641232007688 (sparse_attention).

Data-parallel over batch B=8 across 8 NeuronCores (one batch element per
core). Per core, fp16 on-device with fp32 PSUM accumulation:

  1. depthwise 3x3 convs (BN-folded) on DVE: per tap a 4x-mode
     tensor_scalar mul (host-shifted xp1 copy keeps odd-dx reads
     4B-aligned) + 2x-mode tensor_tensor accumulate, in two row-groups.
  2. layout-B k^T,v^T: per 128-position chunk one [128,1024] PSUM tile
     (2 banks, k and v halves) -> single ACT eviction -> 4 head-pair
     matmuls accumulated DIRECTLY into a long-lived ktv PSUM bank.
  3. channel-major projections qA,kA,vA [512,N] fp16 (PE + ACT evict).
  4. diag = per-head sum(qA*kA) via DVE mult + block-ones matmuls; the
     per-chunk ACT eviction's accum_out gives row partial sums for free;
     m0 broadcast to [128,1] via two tiny matmuls (negated), optional
     1-element DRAM AllReduce between them.
  5. back half per chunk: sb = hselT@diag (PE) evicted with bias=-m0
     (ACT), ew = sb*vA (DVE), oa = ktv_bd@qA - ew (PE, negI matmul,
     DVE eviction), final Wo matmuls + bias via DVE tensor_scalar,
     fp32 DMA out.
"""

import numpy as np

HEADS = 8
DIM = 384
DIM_HEAD = 64
INNER = 512
B = 8
H = W = 56
N = H * W            # 3136
EPS = 1e-5
SCALE = DIM_HEAD ** -0.5
PC = 58              # padded cols/rows
XL = PC * PC + 4     # padded x flat length (+4 OOB slack for shifted reads)
YL = H * PC          # conv output flat length (56 rows x 58 cols)
NCH = 7              # n-chunks of 448 (8 rows) for 448-wide ops
CHW = 448
NB = 25              # n-chunks of 128 for layout-B (24*128 + 64)
CP = 3               # channel partition tiles (384 = 3*128)
HP = 4               # head-dim partition tiles (512 = 4*128)

_CACHE = {}


def _f16(a):
    return np.ascontiguousarray(a, dtype=np.float16)


def _build(reps: int = 1, loop_n=None, no_cc=False):
    import concourse.bacc as bacc
    import concourse.mybir as mybir
    import concourse.tile as tile

    F16 = mybir.dt.float16
    F32 = mybir.dt.float32
    ALU = mybir.AluOpType
    ACTF = mybir.ActivationFunctionType

    nc = bacc.Bacc(None, num_devices=8)

    # ---- DRAM I/O ----
    xp_d = nc.dram_tensor("xp", [DIM, XL], F16, kind="ExternalInput")
    xp1_d = nc.dram_tensor("xp1", [DIM, XL], F16, kind="ExternalInput")
    tq_d = nc.dram_tensor("tq", [DIM, 9], F32, kind="ExternalInput")
    bq_d = nc.dram_tensor("bq", [DIM, 1], F32, kind="ExternalInput")
    tk_d = nc.dram_tensor("tk", [DIM, 9], F32, kind="ExternalInput")
    bk_d = nc.dram_tensor("bk", [DIM, 1], F32, kind="ExternalInput")
    wqT_d = nc.dram_tensor("wqT", [DIM, INNER], F16, kind="ExternalInput")
    wkvT_d = nc.dram_tensor("wkvT", [DIM, 2 * INNER], F16, kind="ExternalInput")
    woT_d = nc.dram_tensor("woT", [INNER, DIM], F16, kind="ExternalInput")
    bo_d = nc.dram_tensor("bo", [DIM, 1], F32, kind="ExternalInput")
    hsel_d = nc.dram_tensor("hsel", [INNER, HEADS], F16, kind="ExternalInput")
    hselT_d = nc.dram_tensor("hselT", [HEADS, INNER], F16, kind="ExternalInput")
    nones8_d = nc.dram_tensor("nones8", [HEADS, 1], F16, kind="ExternalInput")
    ones128r_d = nc.dram_tensor("ones128r", [1, 128], F16, kind="ExternalInput")
    negI_d = nc.dram_tensor("negI", [128, 128], F16, kind="ExternalInput")
    out_d = nc.dram_tensor("out", [DIM, N], F32, kind="ExternalOutput")

    def ch_cols(t, ch):
        return t[:, ch * CHW:(ch + 1) * CHW]

    with tile.TileContext(nc) as tc:
        with (
            tc.tile_pool(name="wsb", bufs=1) as wsb,
            tc.tile_pool(name="xsb", bufs=1) as xsb,
            tc.tile_pool(name="ysb", bufs=1) as ysb,
            tc.tile_pool(name="big", bufs=1) as big,
            tc.tile_pool(name="small", bufs=2) as small,
            tc.tile_pool(name="ev", bufs=3) as ev,
            tc.tile_pool(name="kv_ps", bufs=2, space="PSUM") as kv_ps,
            tc.tile_pool(name="mm_ps", bufs=2, space="PSUM") as mm_ps,
            tc.tile_pool(name="ktv_ps", bufs=1, space="PSUM") as ktv_ps,
            tc.tile_pool(name="dram", bufs=1, space="DRAM") as dram,
        ):
            def emit_body():
                # ---- load padded x (convs gate everything) ----
                x0_t = [xsb.tile([128, XL], F16, tag=f"x0{p}", name=f"x0{p}") for p in range(CP)]
                x1_t = [xsb.tile([128, XL], F16, tag=f"x1{p}", name=f"x1{p}") for p in range(CP)]
                G0E = 26 * PC
                G1S = 24 * PC
                for p in range(CP):
                    cs = slice(p * 128, (p + 1) * 128)
                    nc.sync.dma_start(out=x0_t[p][:, 0:G0E], in_=xp_d[cs, 0:G0E])
                    nc.sync.dma_start(out=x1_t[p][:, 0:G0E], in_=xp1_d[cs, 0:G0E])
                for p in range(CP):
                    cs = slice(p * 128, (p + 1) * 128)
                    nc.sync.dma_start(out=x0_t[p][:, G1S:XL], in_=xp_d[cs, G1S:XL])
                    nc.sync.dma_start(out=x1_t[p][:, G1S:XL], in_=xp1_d[cs, G1S:XL])

                # ---- load weights ----
                wq_t = [wsb.tile([128, INNER], F16, tag=f"wq{p}", name=f"wq{p}") for p in range(CP)]
                wkv_t = [wsb.tile([128, 2 * INNER], F16, tag=f"wkv{p}", name=f"wkv{p}") for p in range(CP)]
                wo_t = [wsb.tile([128, DIM], F16, tag=f"wo{p}", name=f"wo{p}") for p in range(HP)]
                tq_t = [wsb.tile([128, 9], F32, tag=f"tq{p}", name=f"tq{p}") for p in range(CP)]
                bq_t = [wsb.tile([128, 1], F32, tag=f"bq{p}", name=f"bq{p}") for p in range(CP)]
                tk_t = [wsb.tile([128, 9], F32, tag=f"tk{p}", name=f"tk{p}") for p in range(CP)]
                bk_t = [wsb.tile([128, 1], F32, tag=f"bk{p}", name=f"bk{p}") for p in range(CP)]
                bo_t = [wsb.tile([128, 1], F32, tag=f"bo{p}", name=f"bo{p}") for p in range(CP)]
                hsel_t = [wsb.tile([128, HEADS], F16, tag=f"hs{p}", name=f"hs{p}") for p in range(HP)]
                hselT_t = wsb.tile([HEADS, INNER], F16, tag="hsT", name="hsT")
                nones8_t = wsb.tile([HEADS, 1], F16, tag="nones8", name="nones8")
                ones128r_t = wsb.tile([1, 128], F16, tag="ones128r", name="ones128r")
                negI_t = wsb.tile([128, 128], F16, tag="negI", name="negI")
                for p in range(CP):
                    cs = slice(p * 128, (p + 1) * 128)
                    nc.sync.dma_start(out=wq_t[p], in_=wqT_d[cs, :])
                    nc.sync.dma_start(out=wkv_t[p], in_=wkvT_d[cs, :])
                    nc.sync.dma_start(out=tq_t[p], in_=tq_d[cs, :])
                    nc.sync.dma_start(out=bq_t[p], in_=bq_d[cs, :])
                    nc.sync.dma_start(out=tk_t[p], in_=tk_d[cs, :])
                    nc.sync.dma_start(out=bk_t[p], in_=bk_d[cs, :])
                    nc.sync.dma_start(out=bo_t[p], in_=bo_d[cs, :])
                for p in range(HP):
                    cs = slice(p * 128, (p + 1) * 128)
                    nc.sync.dma_start(out=wo_t[p], in_=woT_d[cs, :])
                    nc.sync.dma_start(out=hsel_t[p], in_=hsel_d[cs, :])
                nc.sync.dma_start(out=hselT_t, in_=hselT_d[:, :])
                nc.sync.dma_start(out=nones8_t, in_=nones8_d[:, :])
                nc.sync.dma_start(out=ones128r_t, in_=ones128r_d[:, :])
                nc.sync.dma_start(out=negI_t, in_=negI_d[:, :])

                # ---- depthwise convs (DVE) ----
                RG = ((0, 24), (24, H))

                def conv_group(ys, taps, bias, r0, r1):
                    rows = r1 - r0
                    ve = nc.vector
                    for p in range(CP):
                        y = ys[p]
                        x0v = x0_t[p][:, 0:PC * PC].rearrange("p (a b) -> p a b", b=PC)
                        x1v = x1_t[p][:, 0:PC * PC].rearrange("p (a b) -> p a b", b=PC)
                        first = True
                        for dy in range(3):
                            for dx in range(3):
                                i = dy * 3 + dx
                                if dx == 1:
                                    src = x1v[:, r0 + dy:r1 + dy, 0:W]
                                else:
                                    src = x0v[:, r0 + dy:r1 + dy, dx:dx + W]
                                if first:
                                    ve.tensor_scalar(
                                        out=y[:, r0:r1, :], in0=src,
                                        scalar1=taps[p][:, i:i + 1],
                                        scalar2=bias[p],
                                        op0=ALU.mult, op1=ALU.add)
                                    first = False
                                else:
                                    off = (r0 + dy) * PC + dx
                                    glen = rows * PC
                                    t = ysb.tile([128, glen], F16, tag="tconv",
                                                 name="tconv", bufs=2,
                                                 padded_shape=[128, YL // 2 + PC * 4])
                                    if off % 2 == 0:
                                        tsrc = x0_t[p][:, off:off + glen]
                                    else:
                                        tsrc = x1_t[p][:, off - 1:off - 1 + glen]
                                    ve.tensor_scalar(
                                        out=t, in0=tsrc,
                                        scalar1=taps[p][:, i:i + 1],
                                        scalar2=None, op0=ALU.mult)
                                    tv = t.rearrange("p (a b) -> p a b", b=PC)
                                    ve.tensor_tensor(
                                        out=y[:, r0:r1, :], in0=tv[:, 0:rows, 0:W],
                                        in1=y[:, r0:r1, :], op=ALU.add)

                yq_t = [ysb.tile([128, H, W], F16, tag=f"yq{p}", name=f"yq{p}")
                        for p in range(CP)]
                ykv_t = [ysb.tile([128, H, W], F16, tag=f"ykv{p}", name=f"ykv{p}")
                         for p in range(CP)]
                for (r0, r1) in RG:
                    conv_group(ykv_t, tk_t, bk_t, r0, r1)
                for (r0, r1) in RG:
                    conv_group(yq_t, tq_t, bq_t, r0, r1)
                y_kv = [y.rearrange("p a b -> p (a b)") for y in ykv_t]
                y_q = [y.rearrange("p a b -> p (a b)") for y in yq_t]

                def y_chunk(y, ch):
                    return y[:, ch * CHW:(ch + 1) * CHW]

                # ---- layout-B k^T,v^T + fused ktv accumulation ----
                # Per 128-position chunk: k and v halves each through a
                # [128,512] PSUM tile -> ACT evictions into one fp16 buffer,
                # then 4 head-pair matmuls accumulated directly into 4
                # long-lived single-bank ktv PSUM tiles (one open
                # accumulation group per tile, spanning all chunks).
                ktv_acc = [ktv_ps.tile([128, 128], F32, tag=f"ktva{mp}",
                                       name=f"ktva{mp}") for mp in range(HP)]
                for nb in range(NB):
                    c0 = nb * 128
                    c1 = min(N, c0 + 128)
                    m = c1 - c0
                    kvch = ev.tile([128, 2 * INNER], F16, tag="kvch", name="kvch")
                    for half in range(2):
                        ps = kv_ps.tile([128, INNER], F32, tag="kv", name="kv")
                        for p in range(CP):
                            nc.tensor.matmul(
                                out=ps[0:m, :],
                                lhsT=y_kv[p][:, c0:c1],
                                rhs=wkv_t[p][:, half * INNER:(half + 1) * INNER],
                                start=(p == 0), stop=(p == CP - 1))
                        nc.scalar.copy(
                            out=kvch[0:m, half * INNER:(half + 1) * INNER],
                            in_=ps[0:m, :])
                    for mp in range(HP):
                        ms = slice(mp * 128, (mp + 1) * 128)
                        vs = slice(INNER + mp * 128, INNER + (mp + 1) * 128)
                        nc.tensor.matmul(out=ktv_acc[mp][:, :],
                                         lhsT=kvch[0:m, ms],
                                         rhs=kvch[0:m, vs],
                                         start=(nb == 0), stop=(nb == NB - 1))

                # ---- channel-major projections qA, kA, vA ----
                def proj_A(ys, wts, col0, name):
                    dst = [big.tile([128, N], F16, tag=f"{name}{m}", name=f"{name}{m}") for m in range(HP)]
                    for m in range(HP):
                        for ch in range(NCH):
                            ps = mm_ps.tile([128, CHW], F32, tag="mm", name="mm")
                            for p in range(CP):
                                nc.tensor.matmul(
                                    out=ps[:, :],
                                    lhsT=wts[p][:, col0 + m * 128: col0 + (m + 1) * 128],
                                    rhs=y_chunk(ys[p], ch),
                                    start=(p == 0), stop=(p == CP - 1))
                            nc.scalar.copy(out=ch_cols(dst[m], ch), in_=ps[:, :])
                    return dst

                vA = proj_A(y_kv, wkv_t, INNER, "vA")
                kA = proj_A(y_kv, wkv_t, 0, "kA")
                qA = proj_A(y_q, wq_t, 0, "qA")

                # block-diagonal [ktv_2m, ktv_2m+1] per m-tile (needed only
                # by the back half; emitted late so it doesn't block the
                # projection evictions in the ACT queue)
                ktv_bd = small.tile([128, HP, 128], F16, tag="ktvbd", name="ktvbd")
                nc.vector.memset(ktv_bd, 0.0)
                for mp in range(HP):
                    nc.scalar.copy(out=ktv_bd[0:DIM_HEAD, mp, 0:DIM_HEAD],
                                   in_=ktv_acc[mp][0:DIM_HEAD, 0:DIM_HEAD])
                    nc.scalar.copy(out=ktv_bd[DIM_HEAD:128, mp, DIM_HEAD:128],
                                   in_=ktv_acc[mp][DIM_HEAD:128, DIM_HEAD:128])

                # ---- diag + m0 ----
                diag16 = big.tile([HEADS, N], F16, tag="diag16", name="diag16")
                dcols = small.tile([HEADS, NCH], F32, tag="dcols", name="dcols")
                for ch in range(NCH):
                    dps = mm_ps.tile([HEADS, CHW], F32, tag="mm", name="dps")
                    for m in range(HP):
                        qk = ev.tile([128, CHW], F16, tag="qk", name="qk")
                        nc.vector.tensor_tensor(out=qk, in0=ch_cols(kA[m], ch),
                                                in1=ch_cols(qA[m], ch), op=ALU.mult)
                        nc.tensor.matmul(out=dps[:, :], lhsT=hsel_t[m], rhs=qk,
                                         start=(m == 0), stop=(m == HP - 1))
                    nc.scalar.activation(out=ch_cols(diag16, ch), in_=dps[:, :],
                                         func=ACTF.Identity, bias=0.0, scale=1.0,
                                         accum_out=dcols[:, ch:ch + 1])

                # m0 (negated): dsum = sum(dcols); -m0 = (-1)^T @ dsum;
                # broadcast to [128,1] via a K=1 matmul (no DRAM round trip).
                dsumf = small.tile([HEADS, 1], F32, tag="dsumf", name="dsumf")
                nc.vector.tensor_reduce(out=dsumf, in_=dcols,
                                        axis=mybir.AxisListType.X, op=ALU.add)
                dsum16 = small.tile([HEADS, 1], F16, tag="dsum16", name="dsum16")
                nc.vector.tensor_copy(out=dsum16, in_=dsumf)
                m0_ps = mm_ps.tile([HEADS, CHW], F32, tag="mm", name="m0ps")
                nc.tensor.matmul(out=m0_ps[0:1, 0:1], lhsT=nones8_t, rhs=dsum16,
                                 start=True, stop=True)
                if no_cc:
                    m0r16 = small.tile([1, 1], F16, tag="m0r16", name="m0r16")
                    nc.scalar.copy(out=m0r16, in_=m0_ps[0:1, 0:1])
                else:
                    m0s = small.tile([1, 1], F32, tag="m0s", name="m0s")
                    nc.scalar.copy(out=m0s, in_=m0_ps[0:1, 0:1])
                    cc = dram.tile([1, 1], F32, tag="cc", name="cc")
                    nc.gpsimd.dma_start(out=cc[:, :], in_=m0s)
                    nc.gpsimd.collective_compute(
                        "AllReduce", ALU.add, replica_groups=[list(range(8))],
                        ins=[cc[:, :].opt()], outs=[cc[:, :].opt()])
                    m0r = small.tile([1, 1], F32, tag="m0r", name="m0r")
                    nc.gpsimd.dma_start(out=m0r, in_=cc[:, :])
                    m0r16 = small.tile([1, 1], F16, tag="m0r16", name="m0r16")
                    nc.vector.tensor_copy(out=m0r16, in_=m0r)
                mb_ps = mm_ps.tile([128, CHW], F32, tag="mm", name="mbps")
                nc.tensor.matmul(out=mb_ps[:, 0:1], lhsT=ones128r_t, rhs=m0r16,
                                 start=True, stop=True)
                m0negb = small.tile([128, 1], F32, tag="m0negb", name="m0negb")
                nc.scalar.copy(out=m0negb, in_=mb_ps[:, 0:1])

                # ---- back half, streamed per chunk ----
                for ch in range(NCH):
                    oach = []
                    for m in range(HP):
                        sb_ps = mm_ps.tile([128, CHW], F32, tag="mm", name="sbps")
                        nc.tensor.matmul(out=sb_ps[:, :],
                                         lhsT=hselT_t[:, m * 128:(m + 1) * 128],
                                         rhs=ch_cols(diag16, ch),
                                         start=True, stop=True)
                        # sb16 = diag - m0 (broadcast), fp16
                        sb16 = ev.tile([128, CHW], F16, tag="sb16", name="sb16")
                        nc.scalar.activation(out=sb16, in_=sb_ps[:, :],
                                             func=ACTF.Identity,
                                             bias=m0negb, scale=1.0)
                        ew = ev.tile([128, CHW], F16, tag="ew", name="ew")
                        nc.vector.tensor_tensor(out=ew, in0=sb16,
                                                in1=ch_cols(vA[m], ch), op=ALU.mult)
                        oa_ps = mm_ps.tile([128, CHW], F32, tag="mm", name="oaps")
                        nc.tensor.matmul(out=oa_ps[:, :], lhsT=ktv_bd[:, m, :],
                                         rhs=ch_cols(qA[m], ch),
                                         start=True, stop=False)
                        nc.tensor.matmul(out=oa_ps[:, :], lhsT=negI_t, rhs=ew,
                                         start=False, stop=True)
                        oa = ev.tile([128, CHW], F16, tag=f"oa{m}", name=f"oa{m}")
                        nc.vector.tensor_copy(out=oa, in_=oa_ps[:, :])
                        oach.append(oa)
                    for ot in range(CP):
                        ps = mm_ps.tile([128, CHW], F32, tag="mm", name="fps")
                        for kt in range(HP):
                            nc.tensor.matmul(
                                out=ps[:, :],
                                lhsT=wo_t[kt][:, ot * 128:(ot + 1) * 128],
                                rhs=oach[kt],
                                start=(kt == 0), stop=(kt == HP - 1))
                        of = ev.tile([128, CHW], F32, tag="of", name="of", bufs=2)
                        nc.vector.tensor_scalar(out=of, in0=ps[:, :],
                                                scalar1=bo_t[ot], scalar2=None,
                                                op0=ALU.add)
                        nc.sync.dma_start(
                            out=out_d[ot * 128:(ot + 1) * 128,
                                      ch * CHW:(ch + 1) * CHW],
                            in_=of)

            if loop_n is None:
                for _ in range(reps):
                    emit_body()
            else:
                with tc.For_i(0, loop_n, 1):
                    emit_body()
    nc.finalize()
    return nc


def _get_nc(reps: int = 1, loop_n=None, no_cc=False):
    key = (reps, loop_n, no_cc)
    if key not in _CACHE:
        _CACHE[key] = _build(reps, loop_n, no_cc)
    return _CACHE[key]


def prepare_in_maps(inputs):
    """Host-side preprocessing: fold BN, pad/shift x, transpose weights."""
    x = np.asarray(inputs["x"], np.float32)

    def fold(dw, g, b, m, v):
        inv = np.asarray(g, np.float32) / np.sqrt(np.asarray(v, np.float32) + EPS)
        taps = np.asarray(dw, np.float32)[:, 0].reshape(DIM, 9) * inv[:, None]
        bias = np.asarray(b, np.float32) - np.asarray(m, np.float32) * inv
        return (np.ascontiguousarray(taps, np.float32),
                np.ascontiguousarray(bias[:, None], np.float32))

    tq, bq = fold(inputs["wq_dw"], inputs["wq_bn_g"], inputs["wq_bn_b"],
                  inputs["wq_bn_m"], inputs["wq_bn_v"])
    tk, bk = fold(inputs["wkv_dw"], inputs["wkv_bn_g"], inputs["wkv_bn_b"],
                  inputs["wkv_bn_m"], inputs["wkv_bn_v"])
    wqT = _f16((SCALE * np.asarray(inputs["wq_pw"], np.float32)).T)
    wkvT = _f16(np.asarray(inputs["wkv_pw"], np.float32).T)
    woT = _f16(np.asarray(inputs["wo"], np.float32).T)
    bo = np.ascontiguousarray(np.asarray(inputs["bo"], np.float32)[:, None])
    hsel = _f16(np.repeat(np.eye(HEADS, dtype=np.float32), DIM_HEAD, axis=0))
    hselT = _f16(hsel.T)
    negI = _f16(-np.eye(128, dtype=np.float32))
    nones8 = _f16(-np.ones((HEADS, 1), np.float32))
    ones128r = _f16(np.ones((1, 128), np.float32))

    xpad = np.zeros((B, DIM, PC, PC), np.float16)
    xpad[:, :, 1:1 + H, 1:1 + W] = x.astype(np.float16)
    xflat = np.zeros((B, DIM, XL), np.float16)
    xflat[:, :, :PC * PC] = xpad.reshape(B, DIM, PC * PC)
    xsh = np.zeros_like(xflat)
    xsh[:, :, :XL - 1] = xflat[:, :, 1:]

    shared = dict(tq=tq, bq=bq, tk=tk, bk=bk, wqT=wqT, wkvT=wkvT, woT=woT,
                  bo=bo, hsel=hsel, hselT=hselT, negI=negI, nones8=nones8,
                  ones128r=ones128r)
    return [dict(shared, xp=np.ascontiguousarray(xflat[b]),
                 xp1=np.ascontiguousarray(xsh[b])) for b in range(B)]


def kernel(**inputs) -> np.ndarray:
    from concourse.bass_utils import run_bass_kernel_spmd
    in_maps = prepare_in_maps(inputs)
    nc = _get_nc(1)
    res = run_bass_kernel_spmd(nc, in_maps, list(range(8)))
    out = np.stack([res.results[b]["out"] for b in range(B)])
    return np.ascontiguousarray(out.reshape(B, DIM, H, W).astype(np.float32))


# revision 15
# speedup vs baseline: 7.0761x; 1.0673x over previous
"""Trainium2 Bass kernel for nn_Attention_49641232007688 (sparse_attention).

Data-parallel over batch B=8 across 8 NeuronCores (one batch element per
core). Per core, fp16 on-device with fp32 PSUM accumulation:

  1. depthwise 3x3 convs (BN-folded) on DVE: per tap a 4x-mode
     tensor_scalar mul (host-shifted xp1 copy keeps odd-dx reads
     4B-aligned) + 2x-mode tensor_tensor accumulate, in two row-groups.
  2. layout-B k^T,v^T: per 128-position chunk one [128,1024] PSUM tile
     (2 banks, k and v halves) -> single ACT eviction -> 4 head-pair
     matmuls accumulated DIRECTLY into a long-lived ktv PSUM bank.
  3. channel-major projections qA,kA,vA [512,N] fp16 (PE + ACT evict).
  4. diag = per-head sum(qA*kA) via DVE mult + block-ones matmuls; the
     per-chunk ACT eviction's accum_out gives row partial sums for free;
     m0 broadcast to [128,1] via two tiny matmuls (negated), optional
     1-element DRAM AllReduce between them.
  5. back half per chunk: sb = hselT@diag (PE) evicted with bias=-m0
     (ACT), ew = sb*vA (DVE), oa = ktv_bd@qA - ew (PE, negI matmul,
     DVE eviction), final Wo matmuls + bias via DVE tensor_scalar,
     fp32 DMA out.
"""

import numpy as np

HEADS = 8
DIM = 384
DIM_HEAD = 64
INNER = 512
B = 8
H = W = 56
N = H * W            # 3136
EPS = 1e-5
SCALE = DIM_HEAD ** -0.5
PC = 58              # padded cols/rows
XL = PC * PC + 4     # padded x flat length (+4 OOB slack for shifted reads)
YL = H * PC          # conv output flat length (56 rows x 58 cols)
NCH = 7              # n-chunks of 448 (8 rows) for 448-wide ops
CHW = 448
NB = 25              # n-chunks of 128 for layout-B (24*128 + 64)
CP = 3               # channel partition tiles (384 = 3*128)
HP = 4               # head-dim partition tiles (512 = 4*128)

_CACHE = {}


def _f16(a):
    return np.ascontiguousarray(a, dtype=np.float16)


def _build(reps: int = 1, loop_n=None, no_cc=False):
    import concourse.bacc as bacc
    import concourse.mybir as mybir
    import concourse.tile as tile

    F16 = mybir.dt.float16
    F32 = mybir.dt.float32
    ALU = mybir.AluOpType
    ACTF = mybir.ActivationFunctionType

    nc = bacc.Bacc(None, num_devices=8)

    # ---- DRAM I/O ----
    xp_d = nc.dram_tensor("xp", [DIM, XL], F16, kind="ExternalInput")
    xp1_d = nc.dram_tensor("xp1", [DIM, XL], F16, kind="ExternalInput")
    tq_d = nc.dram_tensor("tq", [DIM, 9], F32, kind="ExternalInput")
    bq_d = nc.dram_tensor("bq", [DIM, 1], F32, kind="ExternalInput")
    tk_d = nc.dram_tensor("tk", [DIM, 9], F32, kind="ExternalInput")
    bk_d = nc.dram_tensor("bk", [DIM, 1], F32, kind="ExternalInput")
    wqT_d = nc.dram_tensor("wqT", [DIM, INNER], F16, kind="ExternalInput")
    wkvT_d = nc.dram_tensor("wkvT", [DIM, 2 * INNER], F16, kind="ExternalInput")
    woT_d = nc.dram_tensor("woT", [INNER, DIM], F16, kind="ExternalInput")
    bo_d = nc.dram_tensor("bo", [DIM, 1], F32, kind="ExternalInput")
    hsel_d = nc.dram_tensor("hsel", [INNER, HEADS], F16, kind="ExternalInput")
    hselT_d = nc.dram_tensor("hselT", [HEADS, INNER], F16, kind="ExternalInput")
    nones8_d = nc.dram_tensor("nones8", [HEADS, 1], F16, kind="ExternalInput")
    ones128r_d = nc.dram_tensor("ones128r", [1, 128], F16, kind="ExternalInput")
    negI_d = nc.dram_tensor("negI", [128, 128], F16, kind="ExternalInput")
    out_d = nc.dram_tensor("out", [DIM, N], F32, kind="ExternalOutput")

    def ch_cols(t, ch):
        return t[:, ch * CHW:(ch + 1) * CHW]

    with tile.TileContext(nc) as tc:
        with (
            tc.tile_pool(name="wsb", bufs=1) as wsb,
            tc.tile_pool(name="xsb", bufs=1) as xsb,
            tc.tile_pool(name="ysb", bufs=1) as ysb,
            tc.tile_pool(name="big", bufs=1) as big,
            tc.tile_pool(name="small", bufs=2) as small,
            tc.tile_pool(name="ev", bufs=3) as ev,
            tc.tile_pool(name="kv_ps", bufs=2, space="PSUM") as kv_ps,
            tc.tile_pool(name="mm_ps", bufs=2, space="PSUM") as mm_ps,
            tc.tile_pool(name="ktv_ps", bufs=1, space="PSUM") as ktv_ps,
            tc.tile_pool(name="dram", bufs=1, space="DRAM") as dram,
        ):
            def emit_body():
                # ---- load padded x (convs gate everything) ----
                x0_t = [xsb.tile([128, XL], F16, tag=f"x0{p}", name=f"x0{p}") for p in range(CP)]
                x1_t = [xsb.tile([128, XL], F16, tag=f"x1{p}", name=f"x1{p}") for p in range(CP)]
                G0E = 26 * PC
                G1S = 24 * PC
                for p in range(CP):
                    cs = slice(p * 128, (p + 1) * 128)
                    nc.sync.dma_start(out=x0_t[p][:, 0:G0E], in_=xp_d[cs, 0:G0E])
                    nc.sync.dma_start(out=x1_t[p][:, 0:G0E], in_=xp1_d[cs, 0:G0E])
                for p in range(CP):
                    cs = slice(p * 128, (p + 1) * 128)
                    nc.sync.dma_start(out=x0_t[p][:, G1S:XL], in_=xp_d[cs, G1S:XL])
                    nc.sync.dma_start(out=x1_t[p][:, G1S:XL], in_=xp1_d[cs, G1S:XL])

                # ---- load weights ----
                wq_t = [wsb.tile([128, INNER], F16, tag=f"wq{p}", name=f"wq{p}") for p in range(CP)]
                wkv_t = [wsb.tile([128, 2 * INNER], F16, tag=f"wkv{p}", name=f"wkv{p}") for p in range(CP)]
                wo_t = [wsb.tile([128, DIM], F16, tag=f"wo{p}", name=f"wo{p}") for p in range(HP)]
                tq_t = [wsb.tile([128, 9], F32, tag=f"tq{p}", name=f"tq{p}") for p in range(CP)]
                bq_t = [wsb.tile([128, 1], F32, tag=f"bq{p}", name=f"bq{p}") for p in range(CP)]
                tk_t = [wsb.tile([128, 9], F32, tag=f"tk{p}", name=f"tk{p}") for p in range(CP)]
                bk_t = [wsb.tile([128, 1], F32, tag=f"bk{p}", name=f"bk{p}") for p in range(CP)]
                bo_t = [wsb.tile([128, 1], F32, tag=f"bo{p}", name=f"bo{p}") for p in range(CP)]
                hsel_t = [wsb.tile([128, HEADS], F16, tag=f"hs{p}", name=f"hs{p}") for p in range(HP)]
                hselT_t = wsb.tile([HEADS, INNER], F16, tag="hsT", name="hsT")
                nones8_t = wsb.tile([HEADS, 1], F16, tag="nones8", name="nones8")
                ones128r_t = wsb.tile([1, 128], F16, tag="ones128r", name="ones128r")
                negI_t = wsb.tile([128, 128], F16, tag="negI", name="negI")
                for p in range(CP):
                    cs = slice(p * 128, (p + 1) * 128)
                    nc.sync.dma_start(out=wq_t[p], in_=wqT_d[cs, :])
                    nc.sync.dma_start(out=wkv_t[p], in_=wkvT_d[cs, :])
                    nc.sync.dma_start(out=tq_t[p], in_=tq_d[cs, :])
                    nc.sync.dma_start(out=bq_t[p], in_=bq_d[cs, :])
                    nc.sync.dma_start(out=tk_t[p], in_=tk_d[cs, :])
                    nc.sync.dma_start(out=bk_t[p], in_=bk_d[cs, :])
                    nc.sync.dma_start(out=bo_t[p], in_=bo_d[cs, :])
                for p in range(HP):
                    cs = slice(p * 128, (p + 1) * 128)
                    nc.sync.dma_start(out=wo_t[p], in_=woT_d[cs, :])
                    nc.sync.dma_start(out=hsel_t[p], in_=hsel_d[cs, :])
                nc.sync.dma_start(out=hselT_t, in_=hselT_d[:, :])
                nc.sync.dma_start(out=nones8_t, in_=nones8_d[:, :])
                nc.sync.dma_start(out=ones128r_t, in_=ones128r_d[:, :])
                nc.sync.dma_start(out=negI_t, in_=negI_d[:, :])

                # ---- depthwise convs (DVE) ----
                RG = ((0, 24), (24, H))

                def conv_group(ys, taps, bias, r0, r1):
                    rows = r1 - r0
                    ve = nc.vector
                    for p in range(CP):
                        y = ys[p]
                        x0v = x0_t[p][:, 0:PC * PC].rearrange("p (a b) -> p a b", b=PC)
                        x1v = x1_t[p][:, 0:PC * PC].rearrange("p (a b) -> p a b", b=PC)
                        first = True
                        for dy in range(3):
                            for dx in range(3):
                                i = dy * 3 + dx
                                if dx == 1:
                                    src = x1v[:, r0 + dy:r1 + dy, 0:W]
                                else:
                                    src = x0v[:, r0 + dy:r1 + dy, dx:dx + W]
                                if first:
                                    ve.tensor_scalar(
                                        out=y[:, r0:r1, :], in0=src,
                                        scalar1=taps[p][:, i:i + 1],
                                        scalar2=bias[p],
                                        op0=ALU.mult, op1=ALU.add)
                                    first = False
                                else:
                                    off = (r0 + dy) * PC + dx
                                    glen = rows * PC
                                    t = ysb.tile([128, glen], F16, tag="tconv",
                                                 name="tconv", bufs=2,
                                                 padded_shape=[128, YL // 2 + PC * 4])
                                    if off % 2 == 0:
                                        tsrc = x0_t[p][:, off:off + glen]
                                    else:
                                        tsrc = x1_t[p][:, off - 1:off - 1 + glen]
                                    ve.tensor_scalar(
                                        out=t, in0=tsrc,
                                        scalar1=taps[p][:, i:i + 1],
                                        scalar2=None, op0=ALU.mult)
                                    tv = t.rearrange("p (a b) -> p a b", b=PC)
                                    ve.tensor_tensor(
                                        out=y[:, r0:r1, :], in0=tv[:, 0:rows, 0:W],
                                        in1=y[:, r0:r1, :], op=ALU.add)

                yq_t = [ysb.tile([128, H, W], F16, tag=f"yq{p}", name=f"yq{p}")
                        for p in range(CP)]
                ykv_t = [ysb.tile([128, H, W], F16, tag=f"ykv{p}", name=f"ykv{p}")
                         for p in range(CP)]
                for (r0, r1) in RG:
                    conv_group(ykv_t, tk_t, bk_t, r0, r1)
                for (r0, r1) in RG:
                    conv_group(yq_t, tq_t, bq_t, r0, r1)
                y_kv = [y.rearrange("p a b -> p (a b)") for y in ykv_t]
                y_q = [y.rearrange("p a b -> p (a b)") for y in yq_t]

                def y_chunk(y, ch):
                    return y[:, ch * CHW:(ch + 1) * CHW]

                # ---- layout-B k^T,v^T + fused ktv accumulation ----
                # Per 128-position chunk: y-chunk-stationary matmuls (k and
                # v halves share each LDWEIGHTS) into two [128,512] PSUM
                # tiles -> ACT evictions into one fp16 buffer, then 4
                # head-pair matmuls accumulated directly into 4 long-lived
                # single-bank ktv PSUM tiles.  The head-pair matmuls for
                # chunk nb are emitted 2 chunks late so the PE never stalls
                # on the eviction; kA/vA projections for the first conv
                # row-group are interleaved mid-loop to cover the window
                # where the conv hasn't yet produced rows for chunk nb+1.
                ktv_acc = [ktv_ps.tile([128, 128], F32, tag=f"ktva{mp}",
                                       name=f"ktva{mp}") for mp in range(HP)]
                kvchs = {}

                def emit_ktv(nb):
                    c0 = nb * 128
                    m = min(N, c0 + 128) - c0
                    kvch = kvchs.pop(nb)
                    for mp in range(HP):
                        ms = slice(mp * 128, (mp + 1) * 128)
                        vs = slice(INNER + mp * 128, INNER + (mp + 1) * 128)
                        nc.tensor.matmul(out=ktv_acc[mp][:, :],
                                         lhsT=kvch[0:m, ms],
                                         rhs=kvch[0:m, vs],
                                         start=(nb == 0), stop=(nb == NB - 1))

                proj_dst = {}

                def proj_chunks(name, ys, wts, col0, chunks):
                    # weight-stationary over chunk pairs: each lhsT slice is
                    # loaded once per pair of 448-col chunks
                    if name not in proj_dst:
                        proj_dst[name] = [
                            big.tile([128, N], F16, tag=f"{name}{m}", name=f"{name}{m}")
                            for m in range(HP)]
                    dst = proj_dst[name]
                    for m in range(HP):
                        i = 0
                        while i < len(chunks):
                            pair = chunks[i:i + 2]
                            pss = [mm_ps.tile([128, CHW], F32, tag="mm", name="mm")
                                   for _ in pair]
                            for p in range(CP):
                                lhsT = wts[p][:, col0 + m * 128: col0 + (m + 1) * 128]
                                for ps, ch in zip(pss, pair):
                                    nc.tensor.matmul(
                                        out=ps[:, :], lhsT=lhsT,
                                        rhs=y_chunk(ys[p], ch),
                                        start=(p == 0), stop=(p == CP - 1))
                            for ps, ch in zip(pss, pair):
                                nc.scalar.copy(out=ch_cols(dst[m], ch), in_=ps[:, :])
                            i += 2
                    return dst

                for nb in range(NB):
                    c0 = nb * 128
                    c1 = min(N, c0 + 128)
                    m = c1 - c0
                    kvch = ev.tile([128, 2 * INNER], F16, tag="kvch", name="kvch")
                    kvchs[nb] = kvch
                    psk = kv_ps.tile([128, INNER], F32, tag="kv", name="kv")
                    psv = kv_ps.tile([128, INNER], F32, tag="kv", name="kv")
                    for p in range(CP):
                        nc.tensor.matmul(
                            out=psk[0:m, :], lhsT=y_kv[p][:, c0:c1],
                            rhs=wkv_t[p][:, 0:INNER],
                            start=(p == 0), stop=(p == CP - 1))
                        nc.tensor.matmul(
                            out=psv[0:m, :], lhsT=y_kv[p][:, c0:c1],
                            rhs=wkv_t[p][:, INNER:2 * INNER],
                            start=(p == 0), stop=(p == CP - 1))
                    nc.scalar.copy(out=kvch[0:m, 0:INNER], in_=psk[0:m, :])
                    nc.scalar.copy(out=kvch[0:m, INNER:2 * INNER], in_=psv[0:m, :])
                    if nb >= 2:
                        emit_ktv(nb - 2)
                    if nb == 10:
                        # conv row-group 0 output (chunks 0-2) keeps the PE
                        # fed while the conv finishes row-group 1
                        proj_chunks("kA", y_kv, wkv_t, 0, [0, 1, 2])
                        proj_chunks("vA", y_kv, wkv_t, INNER, [0, 1, 2])
                emit_ktv(NB - 2)
                emit_ktv(NB - 1)

                kA = proj_chunks("kA", y_kv, wkv_t, 0, [3, 4, 5, 6])
                vA = proj_chunks("vA", y_kv, wkv_t, INNER, [3, 4, 5, 6])
                qA = proj_chunks("qA", y_q, wq_t, 0, [0, 1, 2])
                qA = proj_chunks("qA", y_q, wq_t, 0, [3, 4, 5, 6])

                # block-diagonal [ktv_2m, ktv_2m+1] per m-tile (needed only
                # by the back half; emitted late so it doesn't block the
                # projection evictions in the ACT queue)
                ktv_bd = small.tile([128, HP, 128], F16, tag="ktvbd", name="ktvbd")
                nc.vector.memset(ktv_bd, 0.0)
                for mp in range(HP):
                    nc.scalar.copy(out=ktv_bd[0:DIM_HEAD, mp, 0:DIM_HEAD],
                                   in_=ktv_acc[mp][0:DIM_HEAD, 0:DIM_HEAD])
                    nc.scalar.copy(out=ktv_bd[DIM_HEAD:128, mp, DIM_HEAD:128],
                                   in_=ktv_acc[mp][DIM_HEAD:128, DIM_HEAD:128])

                # ---- diag + m0 ----
                diag16 = big.tile([HEADS, N], F16, tag="diag16", name="diag16")
                dcols = small.tile([HEADS, NCH], F32, tag="dcols", name="dcols")
                for ch in range(NCH):
                    dps = mm_ps.tile([HEADS, CHW], F32, tag="mm", name="dps")
                    qks = []
                    for m in range(HP):
                        qk = ev.tile([128, CHW], F16, tag="qk", name="qk", bufs=4)
                        nc.vector.tensor_tensor(out=qk, in0=ch_cols(kA[m], ch),
                                                in1=ch_cols(qA[m], ch), op=ALU.mult)
                        qks.append(qk)
                    for m in range(HP):
                        nc.tensor.matmul(out=dps[:, :], lhsT=hsel_t[m], rhs=qks[m],
                                         start=(m == 0), stop=(m == HP - 1))
                    nc.scalar.activation(out=ch_cols(diag16, ch), in_=dps[:, :],
                                         func=ACTF.Identity, bias=0.0, scale=1.0,
                                         accum_out=dcols[:, ch:ch + 1])

                # m0 (negated): dsum = sum(dcols); -m0 = (-1)^T @ dsum;
                # broadcast to [128,1] via a K=1 matmul (no DRAM round trip).
                dsumf = small.tile([HEADS, 1], F32, tag="dsumf", name="dsumf")
                nc.vector.tensor_reduce(out=dsumf, in_=dcols,
                                        axis=mybir.AxisListType.X, op=ALU.add)
                dsum16 = small.tile([HEADS, 1], F16, tag="dsum16", name="dsum16")
                nc.vector.tensor_copy(out=dsum16, in_=dsumf)
                m0_ps = mm_ps.tile([HEADS, CHW], F32, tag="mm", name="m0ps")
                nc.tensor.matmul(out=m0_ps[0:1, 0:1], lhsT=nones8_t, rhs=dsum16,
                                 start=True, stop=True)
                if no_cc:
                    m0r16 = small.tile([1, 1], F16, tag="m0r16", name="m0r16")
                    nc.scalar.copy(out=m0r16, in_=m0_ps[0:1, 0:1])
                else:
                    m0s = small.tile([1, 1], F32, tag="m0s", name="m0s")
                    nc.scalar.copy(out=m0s, in_=m0_ps[0:1, 0:1])
                    cc = dram.tile([1, 1], F32, tag="cc", name="cc")
                    nc.gpsimd.dma_start(out=cc[:, :], in_=m0s)
                    nc.gpsimd.collective_compute(
                        "AllReduce", ALU.add, replica_groups=[list(range(8))],
                        ins=[cc[:, :].opt()], outs=[cc[:, :].opt()])
                    m0r = small.tile([1, 1], F32, tag="m0r", name="m0r")
                    nc.gpsimd.dma_start(out=m0r, in_=cc[:, :])
                    m0r16 = small.tile([1, 1], F16, tag="m0r16", name="m0r16")
                    nc.vector.tensor_copy(out=m0r16, in_=m0r)
                mb_ps = mm_ps.tile([128, CHW], F32, tag="mm", name="mbps")
                nc.tensor.matmul(out=mb_ps[:, 0:1], lhsT=ones128r_t, rhs=m0r16,
                                 start=True, stop=True)
                m0negb = small.tile([128, 1], F32, tag="m0negb", name="m0negb")
                nc.scalar.copy(out=m0negb, in_=mb_ps[:, 0:1])

                # ---- back half, streamed per chunk (stage-major so the
                # per-m PE->ACT->DVE->PE round trips pipeline instead of
                # chaining) ----
                for ch in range(NCH):
                    sb16s = []
                    for m in range(HP):
                        sb_ps = mm_ps.tile([128, CHW], F32, tag="mm", name="sbps")
                        nc.tensor.matmul(out=sb_ps[:, :],
                                         lhsT=hselT_t[:, m * 128:(m + 1) * 128],
                                         rhs=ch_cols(diag16, ch),
                                         start=True, stop=True)
                        # sb16 = diag - m0 (broadcast), fp16
                        sb16 = ev.tile([128, CHW], F16, tag="sb16", name="sb16",
                                       bufs=4)
                        nc.scalar.activation(out=sb16, in_=sb_ps[:, :],
                                             func=ACTF.Identity,
                                             bias=m0negb, scale=1.0)
                        sb16s.append(sb16)
                    ews = []
                    for m in range(HP):
                        ew = ev.tile([128, CHW], F16, tag="ew", name="ew", bufs=4)
                        nc.vector.tensor_tensor(out=ew, in0=sb16s[m],
                                                in1=ch_cols(vA[m], ch), op=ALU.mult)
                        ews.append(ew)
                    oach = []
                    for m in range(HP):
                        oa_ps = mm_ps.tile([128, CHW], F32, tag="mm", name="oaps")
                        nc.tensor.matmul(out=oa_ps[:, :], lhsT=ktv_bd[:, m, :],
                                         rhs=ch_cols(qA[m], ch),
                                         start=True, stop=False)
                        nc.tensor.matmul(out=oa_ps[:, :], lhsT=negI_t, rhs=ews[m],
                                         start=False, stop=True)
                        oa = ev.tile([128, CHW], F16, tag=f"oa{m}", name=f"oa{m}",
                                     bufs=2)
                        nc.vector.tensor_copy(out=oa, in_=oa_ps[:, :])
                        oach.append(oa)
                    for ot in range(CP):
                        ps = mm_ps.tile([128, CHW], F32, tag="mm", name="fps")
                        for kt in range(HP):
                            nc.tensor.matmul(
                                out=ps[:, :],
                                lhsT=wo_t[kt][:, ot * 128:(ot + 1) * 128],
                                rhs=oach[kt],
                                start=(kt == 0), stop=(kt == HP - 1))
                        of = ev.tile([128, CHW], F32, tag="of", name="of", bufs=2)
                        nc.vector.tensor_scalar(out=of, in0=ps[:, :],
                                                scalar1=bo_t[ot], scalar2=None,
                                                op0=ALU.add)
                        nc.sync.dma_start(
                            out=out_d[ot * 128:(ot + 1) * 128,
                                      ch * CHW:(ch + 1) * CHW],
                            in_=of)

            if loop_n is None:
                for _ in range(reps):
                    emit_body()
            else:
                with tc.For_i(0, loop_n, 1):
                    emit_body()
    nc.finalize()
    return nc


def _get_nc(reps: int = 1, loop_n=None, no_cc=False):
    key = (reps, loop_n, no_cc)
    if key not in _CACHE:
        _CACHE[key] = _build(reps, loop_n, no_cc)
    return _CACHE[key]


def prepare_in_maps(inputs):
    """Host-side preprocessing: fold BN, pad/shift x, transpose weights."""
    x = np.asarray(inputs["x"], np.float32)

    def fold(dw, g, b, m, v):
        inv = np.asarray(g, np.float32) / np.sqrt(np.asarray(v, np.float32) + EPS)
        taps = np.asarray(dw, np.float32)[:, 0].reshape(DIM, 9) * inv[:, None]
        bias = np.asarray(b, np.float32) - np.asarray(m, np.float32) * inv
        return (np.ascontiguousarray(taps, np.float32),
                np.ascontiguousarray(bias[:, None], np.float32))

    tq, bq = fold(inputs["wq_dw"], inputs["wq_bn_g"], inputs["wq_bn_b"],
                  inputs["wq_bn_m"], inputs["wq_bn_v"])
    tk, bk = fold(inputs["wkv_dw"], inputs["wkv_bn_g"], inputs["wkv_bn_b"],
                  inputs["wkv_bn_m"], inputs["wkv_bn_v"])
    wqT = _f16((SCALE * np.asarray(inputs["wq_pw"], np.float32)).T)
    wkvT = _f16(np.asarray(inputs["wkv_pw"], np.float32).T)
    woT = _f16(np.asarray(inputs["wo"], np.float32).T)
    bo = np.ascontiguousarray(np.asarray(inputs["bo"], np.float32)[:, None])
    hsel = _f16(np.repeat(np.eye(HEADS, dtype=np.float32), DIM_HEAD, axis=0))
    hselT = _f16(hsel.T)
    negI = _f16(-np.eye(128, dtype=np.float32))
    nones8 = _f16(-np.ones((HEADS, 1), np.float32))
    ones128r = _f16(np.ones((1, 128), np.float32))

    xpad = np.zeros((B, DIM, PC, PC), np.float16)
    xpad[:, :, 1:1 + H, 1:1 + W] = x.astype(np.float16)
    xflat = np.zeros((B, DIM, XL), np.float16)
    xflat[:, :, :PC * PC] = xpad.reshape(B, DIM, PC * PC)
    xsh = np.zeros_like(xflat)
    xsh[:, :, :XL - 1] = xflat[:, :, 1:]

    shared = dict(tq=tq, bq=bq, tk=tk, bk=bk, wqT=wqT, wkvT=wkvT, woT=woT,
                  bo=bo, hsel=hsel, hselT=hselT, negI=negI, nones8=nones8,
                  ones128r=ones128r)
    return [dict(shared, xp=np.ascontiguousarray(xflat[b]),
                 xp1=np.ascontiguousarray(xsh[b])) for b in range(B)]


def kernel(**inputs) -> np.ndarray:
    from concourse.bass_utils import run_bass_kernel_spmd
    in_maps = prepare_in_maps(inputs)
    nc = _get_nc(1)
    res = run_bass_kernel_spmd(nc, in_maps, list(range(8)))
    out = np.stack([res.results[b]["out"] for b in range(B)])
    return np.ascontiguousarray(out.reshape(B, DIM, H, W).astype(np.float32))


# revision 19
# speedup vs baseline: 8.1175x; 1.1472x over previous
"""Trainium2 Bass kernel for nn_Attention_49641232007688 (sparse_attention).

Data-parallel over batch B=8 across 8 NeuronCores (one batch element per
core). Per core, fp16 on-device with fp32 PSUM accumulation:

  1. depthwise 3x3 convs (BN-folded) on DVE: per tap a 4x-mode
     tensor_scalar mul (host-shifted xp1 copy keeps odd-dx reads
     4B-aligned) + 2x-mode tensor_tensor accumulate, in two row-groups.
  2. layout-B k^T,v^T: per 128-position chunk one [128,1024] PSUM tile
     (2 banks, k and v halves) -> single ACT eviction -> 4 head-pair
     matmuls accumulated DIRECTLY into a long-lived ktv PSUM bank.
  3. channel-major projections qA,kA,vA [512,N] fp16 (PE + ACT evict).
  4. diag = per-head sum(qA*kA) via DVE mult + block-ones matmuls; the
     per-chunk ACT eviction's accum_out gives row partial sums for free;
     m0 broadcast to [128,1] via two tiny matmuls (negated), optional
     1-element DRAM AllReduce between them.
  5. back half per chunk: sb = hselT@diag (PE) evicted with bias=-m0
     (ACT), ew = sb*vA (DVE), oa = ktv_bd@qA - ew (PE, negI matmul,
     DVE eviction), final Wo matmuls + bias via DVE tensor_scalar,
     fp32 DMA out.
"""

import numpy as np

HEADS = 8
DIM = 384
DIM_HEAD = 64
INNER = 512
B = 8
H = W = 56
N = H * W            # 3136
EPS = 1e-5
SCALE = DIM_HEAD ** -0.5
PC = 58              # padded cols/rows
XL = PC * PC + 4     # padded x flat length (+4 OOB slack for shifted reads)
YL = H * PC          # conv output flat length (56 rows x 58 cols)
NCH = 7              # n-chunks of 448 (8 rows) for 448-wide ops
CHW = 448
NB = 25              # n-chunks of 128 for layout-B (24*128 + 64)
CP = 3               # channel partition tiles (384 = 3*128)
HP = 4               # head-dim partition tiles (512 = 4*128)

_CACHE = {}


def _f16(a):
    return np.ascontiguousarray(a, dtype=np.float16)


def _build(reps: int = 1, loop_n=None, no_cc=False):
    import concourse.bacc as bacc
    import concourse.mybir as mybir
    import concourse.tile as tile

    F16 = mybir.dt.float16
    F32 = mybir.dt.float32
    ALU = mybir.AluOpType
    ACTF = mybir.ActivationFunctionType

    nc = bacc.Bacc(None, num_devices=8)

    # ---- DRAM I/O ----
    xp_d = nc.dram_tensor("xp", [DIM, XL], F16, kind="ExternalInput")
    xp1_d = nc.dram_tensor("xp1", [DIM, XL], F16, kind="ExternalInput")
    tq_d = nc.dram_tensor("tq", [DIM, 9], F32, kind="ExternalInput")
    bq_d = nc.dram_tensor("bq", [DIM, 1], F32, kind="ExternalInput")
    tk_d = nc.dram_tensor("tk", [DIM, 9], F32, kind="ExternalInput")
    bk_d = nc.dram_tensor("bk", [DIM, 1], F32, kind="ExternalInput")
    wqT_d = nc.dram_tensor("wqT", [DIM, INNER], F16, kind="ExternalInput")
    wkvT_d = nc.dram_tensor("wkvT", [DIM, 2 * INNER], F16, kind="ExternalInput")
    woT_d = nc.dram_tensor("woT", [INNER, DIM], F16, kind="ExternalInput")
    bo_d = nc.dram_tensor("bo", [DIM, 1], F32, kind="ExternalInput")
    hsel_d = nc.dram_tensor("hsel", [INNER, HEADS], F16, kind="ExternalInput")
    hselT_d = nc.dram_tensor("hselT", [HEADS, INNER], F16, kind="ExternalInput")
    nones8_d = nc.dram_tensor("nones8", [HEADS, 1], F16, kind="ExternalInput")
    ones128r_d = nc.dram_tensor("ones128r", [1, 128], F16, kind="ExternalInput")
    out_d = nc.dram_tensor("out", [DIM, N], F32, kind="ExternalOutput")

    def ch_cols(t, ch):
        return t[:, ch * CHW:(ch + 1) * CHW]

    with tile.TileContext(nc) as tc:
        with (
            tc.tile_pool(name="wsb", bufs=1) as wsb,
            tc.tile_pool(name="xsb", bufs=1) as xsb,
            tc.tile_pool(name="ysb", bufs=1) as ysb,
            tc.tile_pool(name="big", bufs=1) as big,
            tc.tile_pool(name="small", bufs=2) as small,
            tc.tile_pool(name="ev", bufs=3) as ev,
            tc.tile_pool(name="kv_ps", bufs=2, space="PSUM") as kv_ps,
            tc.tile_pool(name="mm_ps", bufs=2, space="PSUM") as mm_ps,
            tc.tile_pool(name="ktv_ps", bufs=1, space="PSUM") as ktv_ps,
            tc.tile_pool(name="dram", bufs=1, space="DRAM") as dram,
        ):
            def emit_body():
                # ---- load padded x (convs gate everything) ----
                x0_t = [xsb.tile([128, XL], F16, tag=f"x0{p}", name=f"x0{p}") for p in range(CP)]
                x1_t = [xsb.tile([128, XL], F16, tag=f"x1{p}", name=f"x1{p}") for p in range(CP)]
                G0E = 26 * PC
                G1S = 24 * PC
                for p in range(CP):
                    cs = slice(p * 128, (p + 1) * 128)
                    nc.sync.dma_start(out=x0_t[p][:, 0:G0E], in_=xp_d[cs, 0:G0E])
                    nc.sync.dma_start(out=x1_t[p][:, 0:G0E], in_=xp1_d[cs, 0:G0E])
                for p in range(CP):
                    cs = slice(p * 128, (p + 1) * 128)
                    nc.sync.dma_start(out=x0_t[p][:, G1S:XL], in_=xp_d[cs, G1S:XL])
                    nc.sync.dma_start(out=x1_t[p][:, G1S:XL], in_=xp1_d[cs, G1S:XL])

                # ---- load weights ----
                wq_t = [wsb.tile([128, INNER], F16, tag=f"wq{p}", name=f"wq{p}") for p in range(CP)]
                wkv_t = [wsb.tile([128, 2 * INNER], F16, tag=f"wkv{p}", name=f"wkv{p}") for p in range(CP)]
                wo_t = [wsb.tile([128, DIM], F16, tag=f"wo{p}", name=f"wo{p}") for p in range(HP)]
                tq_t = [wsb.tile([128, 9], F32, tag=f"tq{p}", name=f"tq{p}") for p in range(CP)]
                bq_t = [wsb.tile([128, 1], F32, tag=f"bq{p}", name=f"bq{p}") for p in range(CP)]
                tk_t = [wsb.tile([128, 9], F32, tag=f"tk{p}", name=f"tk{p}") for p in range(CP)]
                bk_t = [wsb.tile([128, 1], F32, tag=f"bk{p}", name=f"bk{p}") for p in range(CP)]
                bo_t = [wsb.tile([128, 1], F32, tag=f"bo{p}", name=f"bo{p}") for p in range(CP)]
                hsel_t = [wsb.tile([128, HEADS], F16, tag=f"hs{p}", name=f"hs{p}") for p in range(HP)]
                hselT_t = wsb.tile([HEADS, INNER], F16, tag="hsT", name="hsT")
                nones8_t = wsb.tile([HEADS, 1], F16, tag="nones8", name="nones8")
                ones128r_t = wsb.tile([1, 128], F16, tag="ones128r", name="ones128r")
                for p in range(CP):
                    cs = slice(p * 128, (p + 1) * 128)
                    nc.sync.dma_start(out=wq_t[p], in_=wqT_d[cs, :])
                    nc.sync.dma_start(out=wkv_t[p], in_=wkvT_d[cs, :])
                    nc.sync.dma_start(out=tq_t[p], in_=tq_d[cs, :])
                    nc.sync.dma_start(out=bq_t[p], in_=bq_d[cs, :])
                    nc.sync.dma_start(out=tk_t[p], in_=tk_d[cs, :])
                    nc.sync.dma_start(out=bk_t[p], in_=bk_d[cs, :])
                    nc.sync.dma_start(out=bo_t[p], in_=bo_d[cs, :])
                for p in range(HP):
                    cs = slice(p * 128, (p + 1) * 128)
                    nc.sync.dma_start(out=wo_t[p], in_=woT_d[cs, :])
                    nc.sync.dma_start(out=hsel_t[p], in_=hsel_d[cs, :])
                nc.sync.dma_start(out=hselT_t, in_=hselT_d[:, :])
                nc.sync.dma_start(out=nones8_t, in_=nones8_d[:, :])
                nc.sync.dma_start(out=ones128r_t, in_=ones128r_d[:, :])

                # ---- depthwise convs ----
                # tap (0,0) runs on ACT (scale*x + bias, per-partition APs)
                # for every row-group of both convs, hoisted to the front of
                # the ACT queue where the engine is otherwise idle; the
                # remaining 8 taps are DVE mul(4x)+add(2x) pairs.
                RG = ((0, 24), (24, 40), (40, H))

                yq_t = [ysb.tile([128, H, W], F16, tag=f"yq{p}", name=f"yq{p}")
                        for p in range(CP)]
                ykv_t = [ysb.tile([128, H, W], F16, tag=f"ykv{p}", name=f"ykv{p}")
                         for p in range(CP)]

                def conv_tap0(ys, taps, bias):
                    for (r0, r1) in RG:
                        for p in range(CP):
                            x0v = x0_t[p][:, 0:PC * PC].rearrange(
                                "p (a b) -> p a b", b=PC)
                            nc.scalar.activation(
                                out=ys[p][:, r0:r1, :],
                                in_=x0v[:, r0:r1, 0:W],
                                func=ACTF.Identity,
                                bias=bias[p], scale=taps[p][:, 0:1])

                conv_tap0(ykv_t, tk_t, bk_t)
                conv_tap0(yq_t, tq_t, bq_t)

                def conv_group(ys, taps, r0, r1):
                    rows = r1 - r0
                    ve = nc.vector
                    for p in range(CP):
                        y = ys[p]
                        for dy in range(3):
                            for dx in range(3):
                                i = dy * 3 + dx
                                if i == 0:
                                    continue
                                off = (r0 + dy) * PC + dx
                                glen = rows * PC
                                t = ysb.tile([128, glen], F16, tag="tconv",
                                             name="tconv", bufs=2,
                                             padded_shape=[128, YL // 2 + PC * 4])
                                if off % 2 == 0:
                                    tsrc = x0_t[p][:, off:off + glen]
                                else:
                                    tsrc = x1_t[p][:, off - 1:off - 1 + glen]
                                ve.tensor_scalar(
                                    out=t, in0=tsrc,
                                    scalar1=taps[p][:, i:i + 1],
                                    scalar2=None, op0=ALU.mult)
                                tv = t.rearrange("p (a b) -> p a b", b=PC)
                                ve.tensor_tensor(
                                    out=y[:, r0:r1, :], in0=tv[:, 0:rows, 0:W],
                                    in1=y[:, r0:r1, :], op=ALU.add)

                for (r0, r1) in RG:
                    conv_group(ykv_t, tk_t, r0, r1)
                y_kv = [y.rearrange("p a b -> p (a b)") for y in ykv_t]
                y_q = [y.rearrange("p a b -> p (a b)") for y in yq_t]

                def y_chunk(y, ch):
                    return y[:, ch * CHW:(ch + 1) * CHW]

                # ---- layout-B k^T,v^T + fused ktv accumulation ----
                # Per 128-position chunk: y-chunk-stationary matmuls (k and
                # v halves share each LDWEIGHTS) into two [128,512] PSUM
                # tiles -> ACT evictions into one fp16 buffer, then 4
                # head-pair matmuls accumulated directly into 4 long-lived
                # single-bank ktv PSUM tiles.  The head-pair matmuls for
                # chunk nb are emitted 2 chunks late so the PE never stalls
                # on the eviction; kA/vA projections for the first conv
                # row-group are interleaved mid-loop to cover the window
                # where the conv hasn't yet produced rows for chunk nb+1.
                ktv_acc = [ktv_ps.tile([128, 128], F32, tag=f"ktva{mp}",
                                       name=f"ktva{mp}") for mp in range(HP)]
                kvchs = {}

                def emit_ktv(nb):
                    c0 = nb * 128
                    m = min(N, c0 + 128) - c0
                    kvch = kvchs.pop(nb)
                    for mp in range(HP):
                        ms = slice(mp * 128, (mp + 1) * 128)
                        vs = slice(INNER + mp * 128, INNER + (mp + 1) * 128)
                        nc.tensor.matmul(out=ktv_acc[mp][:, :],
                                         lhsT=kvch[0:m, ms],
                                         rhs=kvch[0:m, vs],
                                         start=(nb == 0), stop=(nb == NB - 1))

                proj_dst = {}

                def proj_chunks(name, ys, wts, col0, chunks, pool=None):
                    # weight-stationary over chunk pairs: each lhsT slice is
                    # loaded once per pair of 448-col chunks
                    pool = pool or mm_ps
                    if name not in proj_dst:
                        proj_dst[name] = [
                            big.tile([128, N], F16, tag=f"{name}{m}", name=f"{name}{m}")
                            for m in range(HP)]
                    dst = proj_dst[name]
                    for m in range(HP):
                        i = 0
                        while i < len(chunks):
                            pair = chunks[i:i + 2]
                            pss = [pool.tile([128, CHW], F32, tag="kv" if pool is kv_ps else "mm", name="mm")
                                   for _ in pair]
                            for p in range(CP):
                                lhsT = wts[p][:, col0 + m * 128: col0 + (m + 1) * 128]
                                for ps, ch in zip(pss, pair):
                                    nc.tensor.matmul(
                                        out=ps[:, :], lhsT=lhsT,
                                        rhs=y_chunk(ys[p], ch),
                                        start=(p == 0), stop=(p == CP - 1))
                            for ps, ch in zip(pss, pair):
                                nc.scalar.copy(out=ch_cols(dst[m], ch), in_=ps[:, :])
                            i += 2
                    return dst

                for nb in range(NB):
                    c0 = nb * 128
                    c1 = min(N, c0 + 128)
                    m = c1 - c0
                    kvch = ev.tile([128, 2 * INNER], F16, tag="kvch", name="kvch")
                    kvchs[nb] = kvch
                    psk = kv_ps.tile([128, INNER], F32, tag="kv", name="kv")
                    psv = kv_ps.tile([128, INNER], F32, tag="kv", name="kv")
                    for p in range(CP):
                        nc.tensor.matmul(
                            out=psk[0:m, :], lhsT=y_kv[p][:, c0:c1],
                            rhs=wkv_t[p][:, 0:INNER],
                            start=(p == 0), stop=(p == CP - 1))
                        nc.tensor.matmul(
                            out=psv[0:m, :], lhsT=y_kv[p][:, c0:c1],
                            rhs=wkv_t[p][:, INNER:2 * INNER],
                            start=(p == 0), stop=(p == CP - 1))
                    nc.scalar.copy(out=kvch[0:m, 0:INNER], in_=psk[0:m, :])
                    nc.scalar.copy(out=kvch[0:m, INNER:2 * INNER], in_=psv[0:m, :])
                    if nb >= 2:
                        emit_ktv(nb - 2)
                    if nb == 10:
                        # conv row-group 0 output (chunks 0-2) keeps the PE
                        # fed while the conv finishes row-group 1; uses the
                        # kv PSUM pool (idle while kvB waits on the conv) so
                        # no WAR cycle with mm-pool users
                        proj_chunks("kA", y_kv, wkv_t, 0, [0, 1, 2], pool=kv_ps)
                        proj_chunks("vA", y_kv, wkv_t, INNER, [0, 1, 2], pool=kv_ps)
                emit_ktv(NB - 2)
                emit_ktv(NB - 1)

                # ---- diag machinery (emitted interleaved with q-conv
                # row-groups so qk lands early in the DVE queue) ----
                diag16 = big.tile([HEADS, N], F16, tag="diag16", name="diag16")
                dcols = small.tile([HEADS, NCH], F32, tag="dcols", name="dcols")

                def emit_diag(chunks):
                    kA, qA = proj_dst["kA"], proj_dst["qA"]
                    for ch in chunks:
                        dps = mm_ps.tile([HEADS, CHW], F32, tag="mm", name="dps")
                        qks = []
                        for m in range(HP):
                            qk = ev.tile([128, CHW], F16, tag="qk", name="qk", bufs=4)
                            nc.vector.tensor_tensor(out=qk, in0=ch_cols(kA[m], ch),
                                                    in1=ch_cols(qA[m], ch),
                                                    op=ALU.mult)
                            qks.append(qk)
                        for m in range(HP):
                            nc.tensor.matmul(out=dps[:, :], lhsT=hsel_t[m],
                                             rhs=qks[m],
                                             start=(m == 0), stop=(m == HP - 1))
                        nc.scalar.activation(out=ch_cols(diag16, ch), in_=dps[:, :],
                                             func=ACTF.Identity, bias=0.0, scale=1.0,
                                             accum_out=dcols[:, ch:ch + 1])

                # q-conv row-groups interleaved with their dependent qA
                # projections and diag chunks
                conv_group(yq_t, tq_t, *RG[0])
                kA = proj_chunks("kA", y_kv, wkv_t, 0, [3, 4, 5, 6])
                vA = proj_chunks("vA", y_kv, wkv_t, INNER, [3, 4, 5, 6])
                qA = proj_chunks("qA", y_q, wq_t, 0, [0, 1, 2])
                emit_diag([0, 1, 2])
                conv_group(yq_t, tq_t, *RG[1])
                qA = proj_chunks("qA", y_q, wq_t, 0, [3, 4])
                emit_diag([3, 4])
                conv_group(yq_t, tq_t, *RG[2])
                qA = proj_chunks("qA", y_q, wq_t, 0, [5, 6])
                emit_diag([5, 6])

                # block-diagonal [ktv_2m, ktv_2m+1] per m-tile
                ktv_bd = small.tile([128, HP, 128], F16, tag="ktvbd", name="ktvbd")
                nc.vector.memset(ktv_bd, 0.0)
                for mp in range(HP):
                    nc.scalar.copy(out=ktv_bd[0:DIM_HEAD, mp, 0:DIM_HEAD],
                                   in_=ktv_acc[mp][0:DIM_HEAD, 0:DIM_HEAD])
                    nc.scalar.copy(out=ktv_bd[DIM_HEAD:128, mp, DIM_HEAD:128],
                                   in_=ktv_acc[mp][DIM_HEAD:128, DIM_HEAD:128])

                # m0 (negated): dsum = sum(dcols); -m0 = (-1)^T @ dsum;
                # broadcast to [128,1] via a K=1 matmul (no DRAM round trip).
                dsumf = small.tile([HEADS, 1], F32, tag="dsumf", name="dsumf")
                nc.vector.tensor_reduce(out=dsumf, in_=dcols,
                                        axis=mybir.AxisListType.X, op=ALU.add)
                dsum16 = small.tile([HEADS, 1], F16, tag="dsum16", name="dsum16")
                nc.vector.tensor_copy(out=dsum16, in_=dsumf)
                m0_ps = mm_ps.tile([HEADS, CHW], F32, tag="mm", name="m0ps")
                nc.tensor.matmul(out=m0_ps[0:1, 0:1], lhsT=nones8_t, rhs=dsum16,
                                 start=True, stop=True)
                if no_cc:
                    m0r16 = small.tile([1, 1], F16, tag="m0r16", name="m0r16")
                    nc.scalar.copy(out=m0r16, in_=m0_ps[0:1, 0:1])
                else:
                    m0s = small.tile([1, 1], F32, tag="m0s", name="m0s")
                    nc.scalar.copy(out=m0s, in_=m0_ps[0:1, 0:1])
                    cc = dram.tile([1, 1], F32, tag="cc", name="cc")
                    nc.gpsimd.dma_start(out=cc[:, :], in_=m0s)
                    nc.gpsimd.collective_compute(
                        "AllReduce", ALU.add, replica_groups=[list(range(8))],
                        ins=[cc[:, :].opt()], outs=[cc[:, :].opt()])
                    m0r = small.tile([1, 1], F32, tag="m0r", name="m0r")
                    nc.gpsimd.dma_start(out=m0r, in_=cc[:, :])
                    m0r16 = small.tile([1, 1], F16, tag="m0r16", name="m0r16")
                    nc.vector.tensor_copy(out=m0r16, in_=m0r)
                mb_ps = mm_ps.tile([128, CHW], F32, tag="mm", name="mbps")
                nc.tensor.matmul(out=mb_ps[:, 0:1], lhsT=ones128r_t, rhs=m0r16,
                                 start=True, stop=True)
                m0negb = small.tile([128, 1], F32, tag="m0negb", name="m0negb")
                nc.scalar.copy(out=m0negb, in_=mb_ps[:, 0:1])

                # ---- back half, streamed per chunk (stage-major so the
                # per-m PE->ACT->DVE->PE round trips pipeline instead of
                # chaining) ----
                for ch in range(NCH):
                    sb16s = []
                    for m in range(HP):
                        sb_ps = mm_ps.tile([128, CHW], F32, tag="mm", name="sbps")
                        nc.tensor.matmul(out=sb_ps[:, :],
                                         lhsT=hselT_t[:, m * 128:(m + 1) * 128],
                                         rhs=ch_cols(diag16, ch),
                                         start=True, stop=True)
                        # sb16 = diag - m0 (broadcast), fp16
                        sb16 = ev.tile([128, CHW], F16, tag="sb16", name="sb16",
                                       bufs=4)
                        nc.scalar.activation(out=sb16, in_=sb_ps[:, :],
                                             func=ACTF.Identity,
                                             bias=m0negb, scale=1.0)
                        sb16s.append(sb16)
                    ews = []
                    for m in range(HP):
                        ew = ev.tile([128, CHW], F16, tag="ew", name="ew", bufs=4)
                        nc.vector.tensor_tensor(out=ew, in0=sb16s[m],
                                                in1=ch_cols(vA[m], ch), op=ALU.mult)
                        ews.append(ew)
                    oach = []
                    for m in range(HP):
                        oa_ps = mm_ps.tile([128, CHW], F32, tag="mm", name="oaps")
                        nc.tensor.matmul(out=oa_ps[:, :], lhsT=ktv_bd[:, m, :],
                                         rhs=ch_cols(qA[m], ch),
                                         start=True, stop=True)
                        # oa = q@ktv - ew, fused into the DVE eviction
                        oa = ev.tile([128, CHW], F16, tag=f"oa{m}", name=f"oa{m}",
                                     bufs=2)
                        nc.vector.tensor_tensor(out=oa, in0=oa_ps[:, :],
                                                in1=ews[m], op=ALU.subtract)
                        oach.append(oa)
                    for ot in range(CP):
                        ps = mm_ps.tile([128, CHW], F32, tag="mm", name="fps")
                        for kt in range(HP):
                            nc.tensor.matmul(
                                out=ps[:, :],
                                lhsT=wo_t[kt][:, ot * 128:(ot + 1) * 128],
                                rhs=oach[kt],
                                start=(kt == 0), stop=(kt == HP - 1))
                        of = ev.tile([128, CHW], F32, tag="of", name="of", bufs=2)
                        nc.scalar.activation(out=of, in_=ps[:, :],
                                             func=ACTF.Identity,
                                             bias=bo_t[ot], scale=1.0)
                        nc.sync.dma_start(
                            out=out_d[ot * 128:(ot + 1) * 128,
                                      ch * CHW:(ch + 1) * CHW],
                            in_=of)

            if loop_n is None:
                for _ in range(reps):
                    emit_body()
            else:
                with tc.For_i(0, loop_n, 1):
                    emit_body()
    nc.finalize()
    return nc


def _get_nc(reps: int = 1, loop_n=None, no_cc=False):
    key = (reps, loop_n, no_cc)
    if key not in _CACHE:
        _CACHE[key] = _build(reps, loop_n, no_cc)
    return _CACHE[key]


def prepare_in_maps(inputs):
    """Host-side preprocessing: fold BN, pad/shift x, transpose weights."""
    x = np.asarray(inputs["x"], np.float32)

    def fold(dw, g, b, m, v):
        inv = np.asarray(g, np.float32) / np.sqrt(np.asarray(v, np.float32) + EPS)
        taps = np.asarray(dw, np.float32)[:, 0].reshape(DIM, 9) * inv[:, None]
        bias = np.asarray(b, np.float32) - np.asarray(m, np.float32) * inv
        return (np.ascontiguousarray(taps, np.float32),
                np.ascontiguousarray(bias[:, None], np.float32))

    tq, bq = fold(inputs["wq_dw"], inputs["wq_bn_g"], inputs["wq_bn_b"],
                  inputs["wq_bn_m"], inputs["wq_bn_v"])
    tk, bk = fold(inputs["wkv_dw"], inputs["wkv_bn_g"], inputs["wkv_bn_b"],
                  inputs["wkv_bn_m"], inputs["wkv_bn_v"])
    wqT = _f16((SCALE * np.asarray(inputs["wq_pw"], np.float32)).T)
    wkvT = _f16(np.asarray(inputs["wkv_pw"], np.float32).T)
    woT = _f16(np.asarray(inputs["wo"], np.float32).T)
    bo = np.ascontiguousarray(np.asarray(inputs["bo"], np.float32)[:, None])
    hsel = _f16(np.repeat(np.eye(HEADS, dtype=np.float32), DIM_HEAD, axis=0))
    hselT = _f16(hsel.T)
    nones8 = _f16(-np.ones((HEADS, 1), np.float32))
    ones128r = _f16(np.ones((1, 128), np.float32))

    xpad = np.zeros((B, DIM, PC, PC), np.float16)
    xpad[:, :, 1:1 + H, 1:1 + W] = x.astype(np.float16)
    xflat = np.zeros((B, DIM, XL), np.float16)
    xflat[:, :, :PC * PC] = xpad.reshape(B, DIM, PC * PC)
    xsh = np.zeros_like(xflat)
    xsh[:, :, :XL - 1] = xflat[:, :, 1:]

    shared = dict(tq=tq, bq=bq, tk=tk, bk=bk, wqT=wqT, wkvT=wkvT, woT=woT,
                  bo=bo, hsel=hsel, hselT=hselT, nones8=nones8,
                  ones128r=ones128r)
    return [dict(shared, xp=np.ascontiguousarray(xflat[b]),
                 xp1=np.ascontiguousarray(xsh[b])) for b in range(B)]


def kernel(**inputs) -> np.ndarray:
    from concourse.bass_utils import run_bass_kernel_spmd
    in_maps = prepare_in_maps(inputs)
    nc = _get_nc(1)
    res = run_bass_kernel_spmd(nc, in_maps, list(range(8)))
    out = np.stack([res.results[b]["out"] for b in range(B)])
    return np.ascontiguousarray(out.reshape(B, DIM, H, W).astype(np.float32))


# revision 29
# speedup vs baseline: 8.2407x; 1.0152x over previous
"""Trainium2 Bass kernel for nn_Attention_49641232007688 (sparse_attention).

Data-parallel over batch B=8 across 8 NeuronCores (one batch element per
core). Per core, fp16 on-device with fp32 PSUM accumulation:

  1. depthwise 3x3 convs (BN-folded) on DVE: per tap a 4x-mode
     tensor_scalar mul (host-shifted xp1 copy keeps odd-dx reads
     4B-aligned) + 2x-mode tensor_tensor accumulate, in two row-groups.
  2. layout-B k^T,v^T: per 128-position chunk one [128,1024] PSUM tile
     (2 banks, k and v halves) -> single ACT eviction -> 4 head-pair
     matmuls accumulated DIRECTLY into a long-lived ktv PSUM bank.
  3. channel-major projections qA,kA,vA [512,N] fp16 (PE + ACT evict).
  4. diag = per-head sum(qA*kA) via DVE mult + block-ones matmuls; the
     per-chunk ACT eviction's accum_out gives row partial sums for free;
     m0 broadcast to [128,1] via two tiny matmuls (negated), optional
     1-element DRAM AllReduce between them.
  5. back half per chunk: sb = hselT@diag (PE) evicted with bias=-m0
     (ACT), ew = sb*vA (DVE), oa = ktv_bd@qA - ew (PE, negI matmul,
     DVE eviction), final Wo matmuls + bias via DVE tensor_scalar,
     fp32 DMA out.
"""

import numpy as np

HEADS = 8
DIM = 384
DIM_HEAD = 64
INNER = 512
B = 8
H = W = 56
N = H * W            # 3136
EPS = 1e-5
SCALE = DIM_HEAD ** -0.5
PC = 58              # padded cols/rows
XL = PC * PC + 4     # padded x flat length (+4 OOB slack for shifted reads)
YL = H * PC          # conv output flat length (56 rows x 58 cols)
NCH = 7              # n-chunks of 448 (8 rows) for 448-wide ops
CHW = 448
NB = 25              # n-chunks of 128 for layout-B (24*128 + 64)
CP = 3               # channel partition tiles (384 = 3*128)
HP = 4               # head-dim partition tiles (512 = 4*128)

_CACHE = {}


def _f16(a):
    return np.ascontiguousarray(a, dtype=np.float16)


def _build(reps: int = 1, loop_n=None, no_cc=False):
    import concourse.bacc as bacc
    import concourse.mybir as mybir
    import concourse.tile as tile

    F16 = mybir.dt.float16
    F32 = mybir.dt.float32
    ALU = mybir.AluOpType
    ACTF = mybir.ActivationFunctionType

    nc = bacc.Bacc(None, num_devices=8)

    # ---- DRAM I/O ----
    xp_d = nc.dram_tensor("xp", [DIM, XL], F16, kind="ExternalInput")
    xp1_d = nc.dram_tensor("xp1", [DIM, XL], F16, kind="ExternalInput")
    tq_d = nc.dram_tensor("tq", [DIM, 9], F32, kind="ExternalInput")
    bq_d = nc.dram_tensor("bq", [DIM, 1], F32, kind="ExternalInput")
    tk_d = nc.dram_tensor("tk", [DIM, 9], F32, kind="ExternalInput")
    bk_d = nc.dram_tensor("bk", [DIM, 1], F32, kind="ExternalInput")
    wqT_d = nc.dram_tensor("wqT", [DIM, INNER], F16, kind="ExternalInput")
    wkvT_d = nc.dram_tensor("wkvT", [DIM, 2 * INNER], F16, kind="ExternalInput")
    woT_d = nc.dram_tensor("woT", [INNER, DIM], F16, kind="ExternalInput")
    bo_d = nc.dram_tensor("bo", [DIM, 1], F32, kind="ExternalInput")
    hsel_d = nc.dram_tensor("hsel", [INNER, HEADS], F16, kind="ExternalInput")
    hselT_d = nc.dram_tensor("hselT", [HEADS, INNER], F16, kind="ExternalInput")
    nones8_d = nc.dram_tensor("nones8", [HEADS, 1], F16, kind="ExternalInput")
    ones128r_d = nc.dram_tensor("ones128r", [1, 128], F16, kind="ExternalInput")
    out_d = nc.dram_tensor("out", [DIM, N], F32, kind="ExternalOutput")

    def ch_cols(t, ch):
        return t[:, ch * CHW:(ch + 1) * CHW]

    with tile.TileContext(nc) as tc:
        with (
            tc.tile_pool(name="wsb", bufs=1) as wsb,
            tc.tile_pool(name="xsb", bufs=1) as xsb,
            tc.tile_pool(name="ysb", bufs=1) as ysb,
            tc.tile_pool(name="big", bufs=1) as big,
            tc.tile_pool(name="small", bufs=2) as small,
            tc.tile_pool(name="ev", bufs=3) as ev,
            tc.tile_pool(name="kv_ps", bufs=2, space="PSUM") as kv_ps,
            tc.tile_pool(name="mm_ps", bufs=2, space="PSUM") as mm_ps,
            tc.tile_pool(name="ktv_ps", bufs=1, space="PSUM") as ktv_ps,
            tc.tile_pool(name="dram", bufs=1, space="DRAM") as dram,
        ):
            def emit_body():
                # ---- load padded x (convs gate everything) ----
                x0_t = [xsb.tile([128, XL], F16, tag=f"x0{p}", name=f"x0{p}") for p in range(CP)]
                x1_t = [xsb.tile([128, XL], F16, tag=f"x1{p}", name=f"x1{p}") for p in range(CP)]
                G0E = 28 * PC
                G1S = 24 * PC
                for p in range(CP):
                    cs = slice(p * 128, (p + 1) * 128)
                    nc.sync.dma_start(out=x0_t[p][:, 0:G0E], in_=xp_d[cs, 0:G0E])
                    nc.sync.dma_start(out=x1_t[p][:, 0:G0E], in_=xp1_d[cs, 0:G0E])
                for p in range(CP):
                    cs = slice(p * 128, (p + 1) * 128)
                    nc.sync.dma_start(out=x0_t[p][:, G1S:XL], in_=xp_d[cs, G1S:XL])
                    nc.sync.dma_start(out=x1_t[p][:, G1S:XL], in_=xp1_d[cs, G1S:XL])

                # ---- load weights ----
                wq_t = [wsb.tile([128, INNER], F16, tag=f"wq{p}", name=f"wq{p}") for p in range(CP)]
                wkv_t = [wsb.tile([128, 2 * INNER], F16, tag=f"wkv{p}", name=f"wkv{p}") for p in range(CP)]
                wo_t = [wsb.tile([128, DIM], F16, tag=f"wo{p}", name=f"wo{p}") for p in range(HP)]
                tq_t = [wsb.tile([128, 9], F32, tag=f"tq{p}", name=f"tq{p}") for p in range(CP)]
                bq_t = [wsb.tile([128, 1], F32, tag=f"bq{p}", name=f"bq{p}") for p in range(CP)]
                tk_t = [wsb.tile([128, 9], F32, tag=f"tk{p}", name=f"tk{p}") for p in range(CP)]
                bk_t = [wsb.tile([128, 1], F32, tag=f"bk{p}", name=f"bk{p}") for p in range(CP)]
                bo_t = [wsb.tile([128, 1], F32, tag=f"bo{p}", name=f"bo{p}") for p in range(CP)]
                hsel_t = [wsb.tile([128, HEADS], F16, tag=f"hs{p}", name=f"hs{p}") for p in range(HP)]
                hselT_t = wsb.tile([HEADS, INNER], F16, tag="hsT", name="hsT")
                nones8_t = wsb.tile([HEADS, 1], F16, tag="nones8", name="nones8")
                ones128r_t = wsb.tile([1, 128], F16, tag="ones128r", name="ones128r")
                for p in range(CP):
                    cs = slice(p * 128, (p + 1) * 128)
                    nc.sync.dma_start(out=wq_t[p], in_=wqT_d[cs, :])
                    nc.sync.dma_start(out=wkv_t[p], in_=wkvT_d[cs, :])
                    nc.sync.dma_start(out=tq_t[p], in_=tq_d[cs, :])
                    nc.sync.dma_start(out=bq_t[p], in_=bq_d[cs, :])
                    nc.sync.dma_start(out=tk_t[p], in_=tk_d[cs, :])
                    nc.sync.dma_start(out=bk_t[p], in_=bk_d[cs, :])
                    nc.sync.dma_start(out=bo_t[p], in_=bo_d[cs, :])
                for p in range(HP):
                    cs = slice(p * 128, (p + 1) * 128)
                    nc.sync.dma_start(out=wo_t[p], in_=woT_d[cs, :])
                    nc.sync.dma_start(out=hsel_t[p], in_=hsel_d[cs, :])
                nc.sync.dma_start(out=hselT_t, in_=hselT_d[:, :])
                nc.sync.dma_start(out=nones8_t, in_=nones8_d[:, :])
                nc.sync.dma_start(out=ones128r_t, in_=ones128r_d[:, :])

                # ---- depthwise convs ----
                # tap (0,0) runs on ACT (scale*x + bias, per-partition APs)
                # for every row-group of both convs, hoisted to the front of
                # the ACT queue where the engine is otherwise idle; the
                # remaining 8 taps are DVE mul(4x)+add(2x) pairs.
                RG = ((0, 24), (24, 40), (40, H))

                yq_t = [ysb.tile([128, H, W], F16, tag=f"yq{p}", name=f"yq{p}")
                        for p in range(CP)]
                ykv_t = [ysb.tile([128, H, W], F16, tag=f"ykv{p}", name=f"ykv{p}")
                         for p in range(CP)]

                def conv_tap0(ys, taps, bias):
                    for (r0, r1) in RG:
                        for p in range(CP):
                            # narrow flat slice so the subtile dep tracker
                            # only waits on the x DMA piece actually read
                            xg = x0_t[p][:, r0 * PC:r1 * PC].rearrange(
                                "p (a b) -> p a b", b=PC)
                            nc.scalar.activation(
                                out=ys[p][:, r0:r1, :],
                                in_=xg[:, :, 0:W],
                                func=ACTF.Identity,
                                bias=bias[p], scale=taps[p][:, 0:1])

                conv_tap0(ykv_t, tk_t, bk_t)
                conv_tap0(yq_t, tq_t, bq_t)

                def conv_group(ys, taps, r0, r1):
                    rows = r1 - r0
                    ve = nc.vector
                    for p in range(CP):
                        y = ys[p]
                        for dy in range(3):
                            for dx in range(3):
                                i = dy * 3 + dx
                                if i == 0:
                                    continue
                                off = (r0 + dy) * PC + dx
                                glen = rows * PC
                                t = ysb.tile([128, glen], F16, tag="tconv",
                                             name="tconv", bufs=2,
                                             padded_shape=[128, YL // 2 + PC * 4])
                                if off % 2 == 0:
                                    tsrc = x0_t[p][:, off:off + glen]
                                else:
                                    tsrc = x1_t[p][:, off - 1:off - 1 + glen]
                                ve.tensor_scalar(
                                    out=t, in0=tsrc,
                                    scalar1=taps[p][:, i:i + 1],
                                    scalar2=None, op0=ALU.mult)
                                tv = t.rearrange("p (a b) -> p a b", b=PC)
                                ve.tensor_tensor(
                                    out=y[:, r0:r1, :], in0=tv[:, 0:rows, 0:W],
                                    in1=y[:, r0:r1, :], op=ALU.add)

                # kv-conv groups 1+2 now; group 3 is emitted after q-conv
                # group 1 (kvB chunks 17-24, which read rows >= 40, are also
                # emitted after it -- program order is the semantics)
                conv_group(ykv_t, tk_t, *RG[0])
                conv_group(ykv_t, tk_t, *RG[1])
                y_kv = [y.rearrange("p a b -> p (a b)") for y in ykv_t]
                y_q = [y.rearrange("p a b -> p (a b)") for y in yq_t]

                def y_chunk(y, ch):
                    return y[:, ch * CHW:(ch + 1) * CHW]

                # ---- layout-B k^T,v^T + fused ktv accumulation ----
                # Per 128-position chunk: y-chunk-stationary matmuls (k and
                # v halves share each LDWEIGHTS) into two [128,512] PSUM
                # tiles -> ACT evictions into one fp16 buffer, then 4
                # head-pair matmuls accumulated directly into 4 long-lived
                # single-bank ktv PSUM tiles.  The head-pair matmuls for
                # chunk nb are emitted 2 chunks late so the PE never stalls
                # on the eviction; kA/vA projections for the first conv
                # row-group are interleaved mid-loop to cover the window
                # where the conv hasn't yet produced rows for chunk nb+1.
                ktv_acc = [ktv_ps.tile([128, 128], F32, tag=f"ktva{mp}",
                                       name=f"ktva{mp}") for mp in range(HP)]
                kvchs = {}

                def emit_ktv(nb):
                    c0 = nb * 128
                    m = min(N, c0 + 128) - c0
                    kvch = kvchs.pop(nb)
                    for mp in range(HP):
                        ms = slice(mp * 128, (mp + 1) * 128)
                        vs = slice(INNER + mp * 128, INNER + (mp + 1) * 128)
                        nc.tensor.matmul(out=ktv_acc[mp][:, :],
                                         lhsT=kvch[0:m, ms],
                                         rhs=kvch[0:m, vs],
                                         start=(nb == 0), stop=(nb == NB - 1))

                proj_dst = {}

                def proj_chunks(name, ys, wts, col0, chunks, pool=None):
                    # weight-stationary over chunk pairs: each lhsT slice is
                    # loaded once per pair of 448-col chunks
                    pool = pool or mm_ps
                    if name not in proj_dst:
                        proj_dst[name] = [
                            big.tile([128, N], F16, tag=f"{name}{m}", name=f"{name}{m}")
                            for m in range(HP)]
                    dst = proj_dst[name]
                    for m in range(HP):
                        i = 0
                        while i < len(chunks):
                            pair = chunks[i:i + 2]
                            pss = [pool.tile([128, CHW], F32, tag="kv" if pool is kv_ps else "mm", name="mm")
                                   for _ in pair]
                            for p in range(CP):
                                lhsT = wts[p][:, col0 + m * 128: col0 + (m + 1) * 128]
                                for ps, ch in zip(pss, pair):
                                    nc.tensor.matmul(
                                        out=ps[:, :], lhsT=lhsT,
                                        rhs=y_chunk(ys[p], ch),
                                        start=(p == 0), stop=(p == CP - 1))
                            for ps, ch in zip(pss, pair):
                                nc.scalar.copy(out=ch_cols(dst[m], ch), in_=ps[:, :])
                            i += 2
                    return dst

                def emit_kvB(nbs):
                    for nb in nbs:
                        c0 = nb * 128
                        c1 = min(N, c0 + 128)
                        m = c1 - c0
                        kvch = ev.tile([128, 2 * INNER], F16, tag="kvch", name="kvch")
                        kvchs[nb] = kvch
                        psk = kv_ps.tile([128, INNER], F32, tag="kv", name="kv")
                        psv = kv_ps.tile([128, INNER], F32, tag="kv", name="kv")
                        for p in range(CP):
                            nc.tensor.matmul(
                                out=psk[0:m, :], lhsT=y_kv[p][:, c0:c1],
                                rhs=wkv_t[p][:, 0:INNER],
                                start=(p == 0), stop=(p == CP - 1))
                            nc.tensor.matmul(
                                out=psv[0:m, :], lhsT=y_kv[p][:, c0:c1],
                                rhs=wkv_t[p][:, INNER:2 * INNER],
                                start=(p == 0), stop=(p == CP - 1))
                        nc.scalar.copy(out=kvch[0:m, 0:INNER], in_=psk[0:m, :])
                        nc.scalar.copy(out=kvch[0:m, INNER:2 * INNER], in_=psv[0:m, :])
                        if nb >= 2:
                            emit_ktv(nb - 2)
                        if nb == 10:
                            # conv row-group 0 output (chunks 0-2) keeps the
                            # PE fed while the conv finishes row-group 1;
                            # uses the kv PSUM pool (idle while kvB waits on
                            # the conv) so no WAR cycle with mm-pool users
                            proj_chunks("kA", y_kv, wkv_t, 0, [0, 1, 2],
                                        pool=kv_ps)
                            proj_chunks("vA", y_kv, wkv_t, INNER, [0, 1, 2],
                                        pool=kv_ps)

                # chunks 0-16 only need conv rows < 40 (groups 1+2)
                emit_kvB(range(17))

                # ---- diag machinery (emitted interleaved with q-conv
                # row-groups so qk lands early in the DVE queue) ----
                diag16 = big.tile([HEADS, N], F16, tag="diag16", name="diag16")
                dcols = small.tile([HEADS, NCH], F32, tag="dcols", name="dcols")

                def emit_diag(chunks):
                    kA, qA = proj_dst["kA"], proj_dst["qA"]
                    for ch in chunks:
                        dps = mm_ps.tile([HEADS, CHW], F32, tag="mm", name="dps")
                        qks = []
                        for m in range(HP):
                            qk = ev.tile([128, CHW], F16, tag="qk", name="qk", bufs=4)
                            nc.vector.tensor_tensor(out=qk, in0=ch_cols(kA[m], ch),
                                                    in1=ch_cols(qA[m], ch),
                                                    op=ALU.mult)
                            qks.append(qk)
                        for m in range(HP):
                            nc.tensor.matmul(out=dps[:, :], lhsT=hsel_t[m],
                                             rhs=qks[m],
                                             start=(m == 0), stop=(m == HP - 1))
                        nc.scalar.activation(out=ch_cols(diag16, ch), in_=dps[:, :],
                                             func=ACTF.Identity, bias=0.0, scale=1.0,
                                             accum_out=dcols[:, ch:ch + 1])

                # remaining conv row-groups interleaved with their dependent
                # projections and diag chunks
                kA = proj_chunks("kA", y_kv, wkv_t, 0, [3, 4])
                vA = proj_chunks("vA", y_kv, wkv_t, INNER, [3, 4])
                conv_group(yq_t, tq_t, *RG[0])
                qA = proj_chunks("qA", y_q, wq_t, 0, [0, 1, 2])
                emit_diag([0, 1, 2])
                conv_group(ykv_t, tk_t, *RG[2])
                emit_kvB(range(17, NB))
                emit_ktv(NB - 2)
                emit_ktv(NB - 1)
                kA = proj_chunks("kA", y_kv, wkv_t, 0, [5, 6])
                vA = proj_chunks("vA", y_kv, wkv_t, INNER, [5, 6])
                conv_group(yq_t, tq_t, *RG[1])
                qA = proj_chunks("qA", y_q, wq_t, 0, [3, 4])
                emit_diag([3, 4])
                conv_group(yq_t, tq_t, *RG[2])
                qA = proj_chunks("qA", y_q, wq_t, 0, [5, 6])
                emit_diag([5, 6])

                # block-diagonal [ktv_2m, ktv_2m+1] per m-tile
                ktv_bd = small.tile([128, HP, 128], F16, tag="ktvbd", name="ktvbd")
                nc.vector.memset(ktv_bd, 0.0)
                for mp in range(HP):
                    nc.scalar.copy(out=ktv_bd[0:DIM_HEAD, mp, 0:DIM_HEAD],
                                   in_=ktv_acc[mp][0:DIM_HEAD, 0:DIM_HEAD])
                    nc.scalar.copy(out=ktv_bd[DIM_HEAD:128, mp, DIM_HEAD:128],
                                   in_=ktv_acc[mp][DIM_HEAD:128, DIM_HEAD:128])

                # m0 (negated): dsum = sum(dcols); -m0 = (-1)^T @ dsum;
                # broadcast to [128,1] via a K=1 matmul (no DRAM round trip).
                dsumf = small.tile([HEADS, 1], F32, tag="dsumf", name="dsumf")
                nc.vector.tensor_reduce(out=dsumf, in_=dcols,
                                        axis=mybir.AxisListType.X, op=ALU.add)
                dsum16 = small.tile([HEADS, 1], F16, tag="dsum16", name="dsum16")
                nc.vector.tensor_copy(out=dsum16, in_=dsumf)
                m0_ps = mm_ps.tile([HEADS, CHW], F32, tag="mm", name="m0ps")
                nc.tensor.matmul(out=m0_ps[0:1, 0:1], lhsT=nones8_t, rhs=dsum16,
                                 start=True, stop=True)
                if no_cc:
                    m0r16 = small.tile([1, 1], F16, tag="m0r16", name="m0r16")
                    nc.scalar.copy(out=m0r16, in_=m0_ps[0:1, 0:1])
                else:
                    m0s = small.tile([1, 1], F32, tag="m0s", name="m0s")
                    nc.scalar.copy(out=m0s, in_=m0_ps[0:1, 0:1])
                    cc = dram.tile([1, 1], F32, tag="cc", name="cc")
                    nc.gpsimd.dma_start(out=cc[:, :], in_=m0s)
                    nc.gpsimd.collective_compute(
                        "AllReduce", ALU.add, replica_groups=[list(range(8))],
                        ins=[cc[:, :].opt()], outs=[cc[:, :].opt()])
                    m0r = small.tile([1, 1], F32, tag="m0r", name="m0r")
                    nc.gpsimd.dma_start(out=m0r, in_=cc[:, :])
                    m0r16 = small.tile([1, 1], F16, tag="m0r16", name="m0r16")
                    nc.vector.tensor_copy(out=m0r16, in_=m0r)
                mb_ps = mm_ps.tile([128, CHW], F32, tag="mm", name="mbps")
                nc.tensor.matmul(out=mb_ps[:, 0:1], lhsT=ones128r_t, rhs=m0r16,
                                 start=True, stop=True)
                m0negb = small.tile([128, 1], F32, tag="m0negb", name="m0negb")
                nc.scalar.copy(out=m0negb, in_=mb_ps[:, 0:1])

                # ---- back half, streamed per chunk (stage-major so the
                # per-m PE->ACT->DVE->PE round trips pipeline instead of
                # chaining) ----
                for ch in range(NCH):
                    sb16s = []
                    for m in range(HP):
                        sb_ps = mm_ps.tile([128, CHW], F32, tag="mm", name="sbps")
                        nc.tensor.matmul(out=sb_ps[:, :],
                                         lhsT=hselT_t[:, m * 128:(m + 1) * 128],
                                         rhs=ch_cols(diag16, ch),
                                         start=True, stop=True)
                        # sb16 = diag - m0 (broadcast), fp16
                        sb16 = ev.tile([128, CHW], F16, tag="sb16", name="sb16",
                                       bufs=4)
                        nc.scalar.activation(out=sb16, in_=sb_ps[:, :],
                                             func=ACTF.Identity,
                                             bias=m0negb, scale=1.0)
                        sb16s.append(sb16)
                    ews = []
                    for m in range(HP):
                        ew = ev.tile([128, CHW], F16, tag="ew", name="ew", bufs=4)
                        nc.vector.tensor_tensor(out=ew, in0=sb16s[m],
                                                in1=ch_cols(vA[m], ch), op=ALU.mult)
                        ews.append(ew)
                    oach = []
                    for m in range(HP):
                        oa_ps = mm_ps.tile([128, CHW], F32, tag="mm", name="oaps")
                        nc.tensor.matmul(out=oa_ps[:, :], lhsT=ktv_bd[:, m, :],
                                         rhs=ch_cols(qA[m], ch),
                                         start=True, stop=True)
                        # oa = q@ktv - ew, fused into the DVE eviction
                        oa = ev.tile([128, CHW], F16, tag=f"oa{m}", name=f"oa{m}",
                                     bufs=2)
                        nc.vector.tensor_tensor(out=oa, in0=oa_ps[:, :],
                                                in1=ews[m], op=ALU.subtract)
                        oach.append(oa)
                    for ot in range(CP):
                        ps = mm_ps.tile([128, CHW], F32, tag="mm", name="fps")
                        for kt in range(HP):
                            nc.tensor.matmul(
                                out=ps[:, :],
                                lhsT=wo_t[kt][:, ot * 128:(ot + 1) * 128],
                                rhs=oach[kt],
                                start=(kt == 0), stop=(kt == HP - 1))
                        of = ev.tile([128, CHW], F32, tag="of", name="of", bufs=2)
                        nc.vector.tensor_scalar(out=of, in0=ps[:, :],
                                                scalar1=bo_t[ot], scalar2=None,
                                                op0=ALU.add)
                        nc.sync.dma_start(
                            out=out_d[ot * 128:(ot + 1) * 128,
                                      ch * CHW:(ch + 1) * CHW],
                            in_=of)

            if loop_n is None:
                for _ in range(reps):
                    emit_body()
            else:
                with tc.For_i(0, loop_n, 1):
                    emit_body()
    nc.finalize()
    return nc


def _get_nc(reps: int = 1, loop_n=None, no_cc=False):
    key = (reps, loop_n, no_cc)
    if key not in _CACHE:
        _CACHE[key] = _build(reps, loop_n, no_cc)
    return _CACHE[key]


def prepare_in_maps(inputs):
    """Host-side preprocessing: fold BN, pad/shift x, transpose weights."""
    x = np.asarray(inputs["x"], np.float32)

    def fold(dw, g, b, m, v):
        inv = np.asarray(g, np.float32) / np.sqrt(np.asarray(v, np.float32) + EPS)
        taps = np.asarray(dw, np.float32)[:, 0].reshape(DIM, 9) * inv[:, None]
        bias = np.asarray(b, np.float32) - np.asarray(m, np.float32) * inv
        return (np.ascontiguousarray(taps, np.float32),
                np.ascontiguousarray(bias[:, None], np.float32))

    tq, bq = fold(inputs["wq_dw"], inputs["wq_bn_g"], inputs["wq_bn_b"],
                  inputs["wq_bn_m"], inputs["wq_bn_v"])
    tk, bk = fold(inputs["wkv_dw"], inputs["wkv_bn_g"], inputs["wkv_bn_b"],
                  inputs["wkv_bn_m"], inputs["wkv_bn_v"])
    wqT = _f16((SCALE * np.asarray(inputs["wq_pw"], np.float32)).T)
    wkvT = _f16(np.asarray(inputs["wkv_pw"], np.float32).T)
    woT = _f16(np.asarray(inputs["wo"], np.float32).T)
    bo = np.ascontiguousarray(np.asarray(inputs["bo"], np.float32)[:, None])
    hsel = _f16(np.repeat(np.eye(HEADS, dtype=np.float32), DIM_HEAD, axis=0))
    hselT = _f16(hsel.T)
    nones8 = _f16(-np.ones((HEADS, 1), np.float32))
    ones128r = _f16(np.ones((1, 128), np.float32))

    xpad = np.zeros((B, DIM, PC, PC), np.float16)
    xpad[:, :, 1:1 + H, 1:1 + W] = x.astype(np.float16)
    xflat = np.zeros((B, DIM, XL), np.float16)
    xflat[:, :, :PC * PC] = xpad.reshape(B, DIM, PC * PC)
    xsh = np.zeros_like(xflat)
    xsh[:, :, :XL - 1] = xflat[:, :, 1:]

    shared = dict(tq=tq, bq=bq, tk=tk, bk=bk, wqT=wqT, wkvT=wkvT, woT=woT,
                  bo=bo, hsel=hsel, hselT=hselT, nones8=nones8,
                  ones128r=ones128r)
    return [dict(shared, xp=np.ascontiguousarray(xflat[b]),
                 xp1=np.ascontiguousarray(xsh[b])) for b in range(B)]


def kernel(**inputs) -> np.ndarray:
    from concourse.bass_utils import run_bass_kernel_spmd
    in_maps = prepare_in_maps(inputs)
    nc = _get_nc(1)
    res = run_bass_kernel_spmd(nc, in_maps, list(range(8)))
    out = np.stack([res.results[b]["out"] for b in range(B)])
    return np.ascontiguousarray(out.reshape(B, DIM, H, W).astype(np.float32))


# revision 30
# speedup vs baseline: 8.3420x; 1.0123x over previous
"""Trainium2 Bass kernel for nn_Attention_49641232007688 (sparse_attention).

Data-parallel over batch B=8 across 8 NeuronCores (one batch element per
core). Per core, fp16 on-device with fp32 PSUM accumulation:

  1. depthwise 3x3 convs (BN-folded) on DVE: per tap a 4x-mode
     tensor_scalar mul (host-shifted xp1 copy keeps odd-dx reads
     4B-aligned) + 2x-mode tensor_tensor accumulate, in two row-groups.
  2. layout-B k^T,v^T: per 128-position chunk one [128,1024] PSUM tile
     (2 banks, k and v halves) -> single ACT eviction -> 4 head-pair
     matmuls accumulated DIRECTLY into a long-lived ktv PSUM bank.
  3. channel-major projections qA,kA,vA [512,N] fp16 (PE + ACT evict).
  4. diag = per-head sum(qA*kA) via DVE mult + block-ones matmuls; the
     per-chunk ACT eviction's accum_out gives row partial sums for free;
     m0 broadcast to [128,1] via two tiny matmuls (negated), optional
     1-element DRAM AllReduce between them.
  5. back half per chunk: sb = hselT@diag (PE) evicted with bias=-m0
     (ACT), ew = sb*vA (DVE), oa = ktv_bd@qA - ew (PE, negI matmul,
     DVE eviction), final Wo matmuls + bias via DVE tensor_scalar,
     fp32 DMA out.
"""

import numpy as np

HEADS = 8
DIM = 384
DIM_HEAD = 64
INNER = 512
B = 8
H = W = 56
N = H * W            # 3136
EPS = 1e-5
SCALE = DIM_HEAD ** -0.5
PC = 58              # padded cols/rows
XL = PC * PC + 4     # padded x flat length (+4 OOB slack for shifted reads)
YL = H * PC          # conv output flat length (56 rows x 58 cols)
NCH = 7              # n-chunks of 448 (8 rows) for 448-wide ops
CHW = 448
NB = 25              # n-chunks of 128 for layout-B (24*128 + 64)
CP = 3               # channel partition tiles (384 = 3*128)
HP = 4               # head-dim partition tiles (512 = 4*128)

_CACHE = {}


def _f16(a):
    return np.ascontiguousarray(a, dtype=np.float16)


def _build(reps: int = 1, loop_n=None, no_cc=False):
    import concourse.bacc as bacc
    import concourse.mybir as mybir
    import concourse.tile as tile

    F16 = mybir.dt.float16
    F32 = mybir.dt.float32
    ALU = mybir.AluOpType
    ACTF = mybir.ActivationFunctionType

    nc = bacc.Bacc(None, num_devices=8)

    # ---- DRAM I/O ----
    xp_d = nc.dram_tensor("xp", [DIM, XL], F16, kind="ExternalInput")
    xp1_d = nc.dram_tensor("xp1", [DIM, XL], F16, kind="ExternalInput")
    tq_d = nc.dram_tensor("tq", [DIM, 9], F32, kind="ExternalInput")
    bq_d = nc.dram_tensor("bq", [DIM, 1], F32, kind="ExternalInput")
    tk_d = nc.dram_tensor("tk", [DIM, 9], F32, kind="ExternalInput")
    bk_d = nc.dram_tensor("bk", [DIM, 1], F32, kind="ExternalInput")
    wqT_d = nc.dram_tensor("wqT", [DIM, INNER], F16, kind="ExternalInput")
    wkvT_d = nc.dram_tensor("wkvT", [DIM, 2 * INNER], F16, kind="ExternalInput")
    woT_d = nc.dram_tensor("woT", [INNER, DIM], F16, kind="ExternalInput")
    bo_d = nc.dram_tensor("bo", [DIM, 1], F32, kind="ExternalInput")
    hsel_d = nc.dram_tensor("hsel", [INNER, HEADS], F16, kind="ExternalInput")
    hselT_d = nc.dram_tensor("hselT", [HEADS, INNER], F16, kind="ExternalInput")
    nones8_d = nc.dram_tensor("nones8", [HEADS, 1], F16, kind="ExternalInput")
    ones128r_d = nc.dram_tensor("ones128r", [1, 128], F16, kind="ExternalInput")
    out_d = nc.dram_tensor("out", [DIM, N], F32, kind="ExternalOutput")

    def ch_cols(t, ch):
        return t[:, ch * CHW:(ch + 1) * CHW]

    with tile.TileContext(nc) as tc:
        with (
            tc.tile_pool(name="wsb", bufs=1) as wsb,
            tc.tile_pool(name="xsb", bufs=1) as xsb,
            tc.tile_pool(name="ysb", bufs=1) as ysb,
            tc.tile_pool(name="big", bufs=1) as big,
            tc.tile_pool(name="small", bufs=2) as small,
            tc.tile_pool(name="ev", bufs=3) as ev,
            tc.tile_pool(name="kv_ps", bufs=2, space="PSUM") as kv_ps,
            tc.tile_pool(name="mm_ps", bufs=2, space="PSUM") as mm_ps,
            tc.tile_pool(name="ktv_ps", bufs=1, space="PSUM") as ktv_ps,
            tc.tile_pool(name="dram", bufs=1, space="DRAM") as dram,
        ):
            def emit_body():
                x0_t = [xsb.tile([128, XL], F16, tag=f"x0{p}", name=f"x0{p}") for p in range(CP)]
                x1_t = [xsb.tile([128, XL], F16, tag=f"x1{p}", name=f"x1{p}") for p in range(CP)]
                wq_t = [wsb.tile([128, INNER], F16, tag=f"wq{p}", name=f"wq{p}") for p in range(CP)]
                wkv_t = [wsb.tile([128, 2 * INNER], F16, tag=f"wkv{p}", name=f"wkv{p}") for p in range(CP)]
                wo_t = [wsb.tile([128, DIM], F16, tag=f"wo{p}", name=f"wo{p}") for p in range(HP)]
                tq_t = [wsb.tile([128, 9], F32, tag=f"tq{p}", name=f"tq{p}") for p in range(CP)]
                bq_t = [wsb.tile([128, 1], F32, tag=f"bq{p}", name=f"bq{p}") for p in range(CP)]
                tk_t = [wsb.tile([128, 9], F32, tag=f"tk{p}", name=f"tk{p}") for p in range(CP)]
                bk_t = [wsb.tile([128, 1], F32, tag=f"bk{p}", name=f"bk{p}") for p in range(CP)]
                bo_t = [wsb.tile([128, 1], F32, tag=f"bo{p}", name=f"bo{p}") for p in range(CP)]
                hsel_t = [wsb.tile([128, HEADS], F16, tag=f"hs{p}", name=f"hs{p}") for p in range(HP)]
                hselT_t = wsb.tile([HEADS, INNER], F16, tag="hsT", name="hsT")
                nones8_t = wsb.tile([HEADS, 1], F16, tag="nones8", name="nones8")
                ones128r_t = wsb.tile([1, 128], F16, tag="ones128r", name="ones128r")

                # tiny conv scalars first (they gate the ACT tap0 ops), then
                # the x pieces, then the big projection weights
                for p in range(CP):
                    cs = slice(p * 128, (p + 1) * 128)
                    nc.sync.dma_start(out=tq_t[p], in_=tq_d[cs, :])
                    nc.sync.dma_start(out=bq_t[p], in_=bq_d[cs, :])
                    nc.sync.dma_start(out=tk_t[p], in_=tk_d[cs, :])
                    nc.sync.dma_start(out=bk_t[p], in_=bk_d[cs, :])
                    nc.sync.dma_start(out=bo_t[p], in_=bo_d[cs, :])
                nc.sync.dma_start(out=hselT_t, in_=hselT_d[:, :])
                nc.sync.dma_start(out=nones8_t, in_=nones8_d[:, :])
                nc.sync.dma_start(out=ones128r_t, in_=ones128r_d[:, :])

                G0E = 28 * PC
                G1S = 24 * PC
                for p in range(CP):
                    cs = slice(p * 128, (p + 1) * 128)
                    nc.sync.dma_start(out=x0_t[p][:, 0:G0E], in_=xp_d[cs, 0:G0E])
                    nc.sync.dma_start(out=x1_t[p][:, 0:G0E], in_=xp1_d[cs, 0:G0E])
                for p in range(CP):
                    cs = slice(p * 128, (p + 1) * 128)
                    nc.sync.dma_start(out=x0_t[p][:, G1S:XL], in_=xp_d[cs, G1S:XL])
                    nc.sync.dma_start(out=x1_t[p][:, G1S:XL], in_=xp1_d[cs, G1S:XL])

                for p in range(CP):
                    cs = slice(p * 128, (p + 1) * 128)
                    nc.sync.dma_start(out=wkv_t[p], in_=wkvT_d[cs, :])
                for p in range(CP):
                    cs = slice(p * 128, (p + 1) * 128)
                    nc.sync.dma_start(out=wq_t[p], in_=wqT_d[cs, :])
                for p in range(HP):
                    cs = slice(p * 128, (p + 1) * 128)
                    nc.sync.dma_start(out=wo_t[p], in_=woT_d[cs, :])
                    nc.sync.dma_start(out=hsel_t[p], in_=hsel_d[cs, :])

                # ---- depthwise convs ----
                # tap (0,0) runs on ACT (scale*x + bias, per-partition APs)
                # for every row-group of both convs, hoisted to the front of
                # the ACT queue where the engine is otherwise idle; the
                # remaining 8 taps are DVE mul(4x)+add(2x) pairs.
                RG = ((0, 24), (24, 40), (40, H))

                yq_t = [ysb.tile([128, H, W], F16, tag=f"yq{p}", name=f"yq{p}")
                        for p in range(CP)]
                ykv_t = [ysb.tile([128, H, W], F16, tag=f"ykv{p}", name=f"ykv{p}")
                         for p in range(CP)]

                def conv_tap0(ys, taps, bias):
                    for (r0, r1) in RG:
                        for p in range(CP):
                            # narrow flat slice so the subtile dep tracker
                            # only waits on the x DMA piece actually read
                            xg = x0_t[p][:, r0 * PC:r1 * PC].rearrange(
                                "p (a b) -> p a b", b=PC)
                            nc.scalar.activation(
                                out=ys[p][:, r0:r1, :],
                                in_=xg[:, :, 0:W],
                                func=ACTF.Identity,
                                bias=bias[p], scale=taps[p][:, 0:1])

                conv_tap0(ykv_t, tk_t, bk_t)
                conv_tap0(yq_t, tq_t, bq_t)

                def conv_group(ys, taps, r0, r1):
                    rows = r1 - r0
                    ve = nc.vector
                    for p in range(CP):
                        y = ys[p]
                        for dy in range(3):
                            for dx in range(3):
                                i = dy * 3 + dx
                                if i == 0:
                                    continue
                                off = (r0 + dy) * PC + dx
                                glen = rows * PC
                                t = ysb.tile([128, glen], F16, tag="tconv",
                                             name="tconv", bufs=2,
                                             padded_shape=[128, YL // 2 + PC * 4])
                                if off % 2 == 0:
                                    tsrc = x0_t[p][:, off:off + glen]
                                else:
                                    tsrc = x1_t[p][:, off - 1:off - 1 + glen]
                                ve.tensor_scalar(
                                    out=t, in0=tsrc,
                                    scalar1=taps[p][:, i:i + 1],
                                    scalar2=None, op0=ALU.mult)
                                tv = t.rearrange("p (a b) -> p a b", b=PC)
                                ve.tensor_tensor(
                                    out=y[:, r0:r1, :], in0=tv[:, 0:rows, 0:W],
                                    in1=y[:, r0:r1, :], op=ALU.add)

                # kv-conv groups 1+2 now; group 3 is emitted after q-conv
                # group 1 (kvB chunks 17-24, which read rows >= 40, are also
                # emitted after it -- program order is the semantics)
                conv_group(ykv_t, tk_t, *RG[0])
                conv_group(ykv_t, tk_t, *RG[1])
                y_kv = [y.rearrange("p a b -> p (a b)") for y in ykv_t]
                y_q = [y.rearrange("p a b -> p (a b)") for y in yq_t]

                def y_chunk(y, ch):
                    return y[:, ch * CHW:(ch + 1) * CHW]

                # ---- layout-B k^T,v^T + fused ktv accumulation ----
                # Per 128-position chunk: y-chunk-stationary matmuls (k and
                # v halves share each LDWEIGHTS) into two [128,512] PSUM
                # tiles -> ACT evictions into one fp16 buffer, then 4
                # head-pair matmuls accumulated directly into 4 long-lived
                # single-bank ktv PSUM tiles.  The head-pair matmuls for
                # chunk nb are emitted 2 chunks late so the PE never stalls
                # on the eviction; kA/vA projections for the first conv
                # row-group are interleaved mid-loop to cover the window
                # where the conv hasn't yet produced rows for chunk nb+1.
                ktv_acc = [ktv_ps.tile([128, 128], F32, tag=f"ktva{mp}",
                                       name=f"ktva{mp}") for mp in range(HP)]
                kvchs = {}

                def emit_ktv(nb):
                    c0 = nb * 128
                    m = min(N, c0 + 128) - c0
                    kvch = kvchs.pop(nb)
                    for mp in range(HP):
                        ms = slice(mp * 128, (mp + 1) * 128)
                        vs = slice(INNER + mp * 128, INNER + (mp + 1) * 128)
                        nc.tensor.matmul(out=ktv_acc[mp][:, :],
                                         lhsT=kvch[0:m, ms],
                                         rhs=kvch[0:m, vs],
                                         start=(nb == 0), stop=(nb == NB - 1))

                proj_dst = {}

                def proj_chunks(name, ys, wts, col0, chunks, pool=None):
                    # weight-stationary over chunk pairs: each lhsT slice is
                    # loaded once per pair of 448-col chunks
                    pool = pool or mm_ps
                    if name not in proj_dst:
                        proj_dst[name] = [
                            big.tile([128, N], F16, tag=f"{name}{m}", name=f"{name}{m}")
                            for m in range(HP)]
                    dst = proj_dst[name]
                    for m in range(HP):
                        i = 0
                        while i < len(chunks):
                            pair = chunks[i:i + 2]
                            pss = [pool.tile([128, CHW], F32, tag="kv" if pool is kv_ps else "mm", name="mm")
                                   for _ in pair]
                            for p in range(CP):
                                lhsT = wts[p][:, col0 + m * 128: col0 + (m + 1) * 128]
                                for ps, ch in zip(pss, pair):
                                    nc.tensor.matmul(
                                        out=ps[:, :], lhsT=lhsT,
                                        rhs=y_chunk(ys[p], ch),
                                        start=(p == 0), stop=(p == CP - 1))
                            for ps, ch in zip(pss, pair):
                                nc.scalar.copy(out=ch_cols(dst[m], ch), in_=ps[:, :])
                            i += 2
                    return dst

                def emit_kvB(nbs):
                    for nb in nbs:
                        c0 = nb * 128
                        c1 = min(N, c0 + 128)
                        m = c1 - c0
                        kvch = ev.tile([128, 2 * INNER], F16, tag="kvch", name="kvch")
                        kvchs[nb] = kvch
                        psk = kv_ps.tile([128, INNER], F32, tag="kv", name="kv")
                        psv = kv_ps.tile([128, INNER], F32, tag="kv", name="kv")
                        for p in range(CP):
                            nc.tensor.matmul(
                                out=psk[0:m, :], lhsT=y_kv[p][:, c0:c1],
                                rhs=wkv_t[p][:, 0:INNER],
                                start=(p == 0), stop=(p == CP - 1))
                            nc.tensor.matmul(
                                out=psv[0:m, :], lhsT=y_kv[p][:, c0:c1],
                                rhs=wkv_t[p][:, INNER:2 * INNER],
                                start=(p == 0), stop=(p == CP - 1))
                        nc.scalar.copy(out=kvch[0:m, 0:INNER], in_=psk[0:m, :])
                        nc.scalar.copy(out=kvch[0:m, INNER:2 * INNER], in_=psv[0:m, :])
                        if nb >= 2:
                            emit_ktv(nb - 2)
                        if nb == 10:
                            # conv row-group 0 output (chunks 0-2) keeps the
                            # PE fed while the conv finishes row-group 1;
                            # uses the kv PSUM pool (idle while kvB waits on
                            # the conv) so no WAR cycle with mm-pool users
                            proj_chunks("kA", y_kv, wkv_t, 0, [0, 1, 2],
                                        pool=kv_ps)
                            proj_chunks("vA", y_kv, wkv_t, INNER, [0, 1, 2],
                                        pool=kv_ps)

                # chunks 0-16 only need conv rows < 40 (groups 1+2)
                emit_kvB(range(17))

                # ---- diag machinery (emitted interleaved with q-conv
                # row-groups so qk lands early in the DVE queue) ----
                diag16 = big.tile([HEADS, N], F16, tag="diag16", name="diag16")
                dcols = small.tile([HEADS, NCH], F32, tag="dcols", name="dcols")

                def emit_diag(chunks):
                    kA, qA = proj_dst["kA"], proj_dst["qA"]
                    for ch in chunks:
                        dps = mm_ps.tile([HEADS, CHW], F32, tag="mm", name="dps")
                        qks = []
                        for m in range(HP):
                            qk = ev.tile([128, CHW], F16, tag="qk", name="qk", bufs=4)
                            nc.vector.tensor_tensor(out=qk, in0=ch_cols(kA[m], ch),
                                                    in1=ch_cols(qA[m], ch),
                                                    op=ALU.mult)
                            qks.append(qk)
                        for m in range(HP):
                            nc.tensor.matmul(out=dps[:, :], lhsT=hsel_t[m],
                                             rhs=qks[m],
                                             start=(m == 0), stop=(m == HP - 1))
                        nc.scalar.activation(out=ch_cols(diag16, ch), in_=dps[:, :],
                                             func=ACTF.Identity, bias=0.0, scale=1.0,
                                             accum_out=dcols[:, ch:ch + 1])

                # remaining conv row-groups interleaved with their dependent
                # projections and diag chunks
                kA = proj_chunks("kA", y_kv, wkv_t, 0, [3, 4])
                vA = proj_chunks("vA", y_kv, wkv_t, INNER, [3, 4])
                conv_group(yq_t, tq_t, *RG[0])
                qA = proj_chunks("qA", y_q, wq_t, 0, [0, 1, 2])
                emit_diag([0, 1, 2])
                conv_group(ykv_t, tk_t, *RG[2])
                emit_kvB(range(17, NB))
                emit_ktv(NB - 2)
                emit_ktv(NB - 1)
                kA = proj_chunks("kA", y_kv, wkv_t, 0, [5, 6])
                vA = proj_chunks("vA", y_kv, wkv_t, INNER, [5, 6])
                conv_group(yq_t, tq_t, *RG[1])
                qA = proj_chunks("qA", y_q, wq_t, 0, [3, 4])
                emit_diag([3, 4])
                conv_group(yq_t, tq_t, *RG[2])
                qA = proj_chunks("qA", y_q, wq_t, 0, [5, 6])
                emit_diag([5, 6])

                # block-diagonal [ktv_2m, ktv_2m+1] per m-tile
                ktv_bd = small.tile([128, HP, 128], F16, tag="ktvbd", name="ktvbd")
                nc.vector.memset(ktv_bd, 0.0)
                for mp in range(HP):
                    nc.scalar.copy(out=ktv_bd[0:DIM_HEAD, mp, 0:DIM_HEAD],
                                   in_=ktv_acc[mp][0:DIM_HEAD, 0:DIM_HEAD])
                    nc.scalar.copy(out=ktv_bd[DIM_HEAD:128, mp, DIM_HEAD:128],
                                   in_=ktv_acc[mp][DIM_HEAD:128, DIM_HEAD:128])

                # m0 (negated): dsum = sum(dcols); -m0 = (-1)^T @ dsum;
                # broadcast to [128,1] via a K=1 matmul (no DRAM round trip).
                dsumf = small.tile([HEADS, 1], F32, tag="dsumf", name="dsumf")
                nc.vector.tensor_reduce(out=dsumf, in_=dcols,
                                        axis=mybir.AxisListType.X, op=ALU.add)
                dsum16 = small.tile([HEADS, 1], F16, tag="dsum16", name="dsum16")
                nc.vector.tensor_copy(out=dsum16, in_=dsumf)
                m0_ps = mm_ps.tile([HEADS, CHW], F32, tag="mm", name="m0ps")
                nc.tensor.matmul(out=m0_ps[0:1, 0:1], lhsT=nones8_t, rhs=dsum16,
                                 start=True, stop=True)
                if no_cc:
                    m0r16 = small.tile([1, 1], F16, tag="m0r16", name="m0r16")
                    nc.scalar.copy(out=m0r16, in_=m0_ps[0:1, 0:1])
                else:
                    m0s = small.tile([1, 1], F32, tag="m0s", name="m0s")
                    nc.scalar.copy(out=m0s, in_=m0_ps[0:1, 0:1])
                    cc = dram.tile([1, 1], F32, tag="cc", name="cc")
                    nc.gpsimd.dma_start(out=cc[:, :], in_=m0s)
                    nc.gpsimd.collective_compute(
                        "AllReduce", ALU.add, replica_groups=[list(range(8))],
                        ins=[cc[:, :].opt()], outs=[cc[:, :].opt()])
                    m0r = small.tile([1, 1], F32, tag="m0r", name="m0r")
                    nc.gpsimd.dma_start(out=m0r, in_=cc[:, :])
                    m0r16 = small.tile([1, 1], F16, tag="m0r16", name="m0r16")
                    nc.vector.tensor_copy(out=m0r16, in_=m0r)
                mb_ps = mm_ps.tile([128, CHW], F32, tag="mm", name="mbps")
                nc.tensor.matmul(out=mb_ps[:, 0:1], lhsT=ones128r_t, rhs=m0r16,
                                 start=True, stop=True)
                m0negb = small.tile([128, 1], F32, tag="m0negb", name="m0negb")
                nc.scalar.copy(out=m0negb, in_=mb_ps[:, 0:1])

                # ---- back half, streamed per chunk (stage-major so the
                # per-m PE->ACT->DVE->PE round trips pipeline instead of
                # chaining) ----
                for ch in range(NCH):
                    sb16s = []
                    for m in range(HP):
                        sb_ps = mm_ps.tile([128, CHW], F32, tag="mm", name="sbps")
                        nc.tensor.matmul(out=sb_ps[:, :],
                                         lhsT=hselT_t[:, m * 128:(m + 1) * 128],
                                         rhs=ch_cols(diag16, ch),
                                         start=True, stop=True)
                        # sb16 = diag - m0 (broadcast), fp16
                        sb16 = ev.tile([128, CHW], F16, tag="sb16", name="sb16",
                                       bufs=4)
                        nc.scalar.activation(out=sb16, in_=sb_ps[:, :],
                                             func=ACTF.Identity,
                                             bias=m0negb, scale=1.0)
                        sb16s.append(sb16)
                    ews = []
                    for m in range(HP):
                        ew = ev.tile([128, CHW], F16, tag="ew", name="ew", bufs=4)
                        nc.vector.tensor_tensor(out=ew, in0=sb16s[m],
                                                in1=ch_cols(vA[m], ch), op=ALU.mult)
                        ews.append(ew)
                    oach = []
                    for m in range(HP):
                        oa_ps = mm_ps.tile([128, CHW], F32, tag="mm", name="oaps")
                        nc.tensor.matmul(out=oa_ps[:, :], lhsT=ktv_bd[:, m, :],
                                         rhs=ch_cols(qA[m], ch),
                                         start=True, stop=True)
                        # oa = q@ktv - ew, fused into the DVE eviction
                        oa = ev.tile([128, CHW], F16, tag=f"oa{m}", name=f"oa{m}",
                                     bufs=2)
                        nc.vector.tensor_tensor(out=oa, in0=oa_ps[:, :],
                                                in1=ews[m], op=ALU.subtract)
                        oach.append(oa)
                    for ot in range(CP):
                        ps = mm_ps.tile([128, CHW], F32, tag="mm", name="fps")
                        for kt in range(HP):
                            nc.tensor.matmul(
                                out=ps[:, :],
                                lhsT=wo_t[kt][:, ot * 128:(ot + 1) * 128],
                                rhs=oach[kt],
                                start=(kt == 0), stop=(kt == HP - 1))
                        of = ev.tile([128, CHW], F32, tag="of", name="of", bufs=2)
                        nc.vector.tensor_scalar(out=of, in0=ps[:, :],
                                                scalar1=bo_t[ot], scalar2=None,
                                                op0=ALU.add)
                        nc.sync.dma_start(
                            out=out_d[ot * 128:(ot + 1) * 128,
                                      ch * CHW:(ch + 1) * CHW],
                            in_=of)

            if loop_n is None:
                for _ in range(reps):
                    emit_body()
            else:
                with tc.For_i(0, loop_n, 1):
                    emit_body()
    nc.finalize()
    return nc


def _get_nc(reps: int = 1, loop_n=None, no_cc=False):
    key = (reps, loop_n, no_cc)
    if key not in _CACHE:
        _CACHE[key] = _build(reps, loop_n, no_cc)
    return _CACHE[key]


def prepare_in_maps(inputs):
    """Host-side preprocessing: fold BN, pad/shift x, transpose weights."""
    x = np.asarray(inputs["x"], np.float32)

    def fold(dw, g, b, m, v):
        inv = np.asarray(g, np.float32) / np.sqrt(np.asarray(v, np.float32) + EPS)
        taps = np.asarray(dw, np.float32)[:, 0].reshape(DIM, 9) * inv[:, None]
        bias = np.asarray(b, np.float32) - np.asarray(m, np.float32) * inv
        return (np.ascontiguousarray(taps, np.float32),
                np.ascontiguousarray(bias[:, None], np.float32))

    tq, bq = fold(inputs["wq_dw"], inputs["wq_bn_g"], inputs["wq_bn_b"],
                  inputs["wq_bn_m"], inputs["wq_bn_v"])
    tk, bk = fold(inputs["wkv_dw"], inputs["wkv_bn_g"], inputs["wkv_bn_b"],
                  inputs["wkv_bn_m"], inputs["wkv_bn_v"])
    wqT = _f16((SCALE * np.asarray(inputs["wq_pw"], np.float32)).T)
    wkvT = _f16(np.asarray(inputs["wkv_pw"], np.float32).T)
    woT = _f16(np.asarray(inputs["wo"], np.float32).T)
    bo = np.ascontiguousarray(np.asarray(inputs["bo"], np.float32)[:, None])
    hsel = _f16(np.repeat(np.eye(HEADS, dtype=np.float32), DIM_HEAD, axis=0))
    hselT = _f16(hsel.T)
    nones8 = _f16(-np.ones((HEADS, 1), np.float32))
    ones128r = _f16(np.ones((1, 128), np.float32))

    xpad = np.zeros((B, DIM, PC, PC), np.float16)
    xpad[:, :, 1:1 + H, 1:1 + W] = x.astype(np.float16)
    xflat = np.zeros((B, DIM, XL), np.float16)
    xflat[:, :, :PC * PC] = xpad.reshape(B, DIM, PC * PC)
    xsh = np.zeros_like(xflat)
    xsh[:, :, :XL - 1] = xflat[:, :, 1:]

    shared = dict(tq=tq, bq=bq, tk=tk, bk=bk, wqT=wqT, wkvT=wkvT, woT=woT,
                  bo=bo, hsel=hsel, hselT=hselT, nones8=nones8,
                  ones128r=ones128r)
    return [dict(shared, xp=np.ascontiguousarray(xflat[b]),
                 xp1=np.ascontiguousarray(xsh[b])) for b in range(B)]


def kernel(**inputs) -> np.ndarray:
    from concourse.bass_utils import run_bass_kernel_spmd
    in_maps = prepare_in_maps(inputs)
    nc = _get_nc(1)
    res = run_bass_kernel_spmd(nc, in_maps, list(range(8)))
    out = np.stack([res.results[b]["out"] for b in range(B)])
    return np.ascontiguousarray(out.reshape(B, DIM, H, W).astype(np.float32))


# revision 32
# speedup vs baseline: 9.2062x; 1.1036x over previous
"""Trainium2 Bass kernel for nn_Attention_49641232007688 (sparse_attention).

Data-parallel over batch B=8 across 8 NeuronCores (one batch element per
core). Per core, fp16 on-device with fp32 PSUM accumulation:

  1. depthwise 3x3 convs (BN-folded) on DVE: per tap a 4x-mode
     tensor_scalar mul (host-shifted xp1 copy keeps odd-dx reads
     4B-aligned) + 2x-mode tensor_tensor accumulate, in two row-groups.
  2. layout-B k^T,v^T: per 128-position chunk one [128,1024] PSUM tile
     (2 banks, k and v halves) -> single ACT eviction -> 4 head-pair
     matmuls accumulated DIRECTLY into a long-lived ktv PSUM bank.
  3. channel-major projections qA,kA,vA [512,N] fp16 (PE + ACT evict).
  4. diag = per-head sum(qA*kA) via DVE mult + block-ones matmuls; the
     per-chunk ACT eviction's accum_out gives row partial sums for free;
     m0 broadcast to [128,1] via two tiny matmuls (negated), optional
     1-element DRAM AllReduce between them.
  5. back half per chunk: sb = hselT@diag (PE) evicted with bias=-m0
     (ACT), ew = sb*vA (DVE), oa = ktv_bd@qA - ew (PE, negI matmul,
     DVE eviction), final Wo matmuls + bias via DVE tensor_scalar,
     fp32 DMA out.
"""

import numpy as np

HEADS = 8
DIM = 384
DIM_HEAD = 64
INNER = 512
B = 8
H = W = 56
N = H * W            # 3136
EPS = 1e-5
SCALE = DIM_HEAD ** -0.5
PC = 58              # padded cols/rows
XL = PC * PC + 4     # padded x flat length (+4 OOB slack for shifted reads)
YL = H * PC          # conv output flat length (56 rows x 58 cols)
NCH = 7              # n-chunks of 448 (8 rows) for 448-wide ops
CHW = 448
NB = 25              # n-chunks of 128 for layout-B (24*128 + 64)
CP = 3               # channel partition tiles (384 = 3*128)
HP = 4               # head-dim partition tiles (512 = 4*128)

_CACHE = {}


def _f16(a):
    return np.ascontiguousarray(a, dtype=np.float16)


def _build(reps: int = 1, loop_n=None, no_cc=False):
    import concourse.bacc as bacc
    import concourse.mybir as mybir
    import concourse.tile as tile

    F16 = mybir.dt.float16
    F32 = mybir.dt.float32
    ALU = mybir.AluOpType
    ACTF = mybir.ActivationFunctionType

    nc = bacc.Bacc(None, num_devices=8)

    # ---- DRAM I/O ----
    xp_d = nc.dram_tensor("xp", [DIM, XL], F16, kind="ExternalInput")
    xp1_d = nc.dram_tensor("xp1", [DIM, XL], F16, kind="ExternalInput")
    tq_d = nc.dram_tensor("tq", [DIM, 9], F32, kind="ExternalInput")
    bq_d = nc.dram_tensor("bq", [DIM, 1], F32, kind="ExternalInput")
    tk_d = nc.dram_tensor("tk", [DIM, 9], F32, kind="ExternalInput")
    bk_d = nc.dram_tensor("bk", [DIM, 1], F32, kind="ExternalInput")
    wqT_d = nc.dram_tensor("wqT", [DIM, INNER], F16, kind="ExternalInput")
    wkvT_d = nc.dram_tensor("wkvT", [DIM, 2 * INNER], F16, kind="ExternalInput")
    woT_d = nc.dram_tensor("woT", [INNER, DIM], F16, kind="ExternalInput")
    bo_d = nc.dram_tensor("bo", [DIM, 1], F32, kind="ExternalInput")
    hsel_d = nc.dram_tensor("hsel", [INNER, HEADS], F16, kind="ExternalInput")
    hselT_d = nc.dram_tensor("hselT", [HEADS, INNER], F16, kind="ExternalInput")
    nones8_d = nc.dram_tensor("nones8", [HEADS, 1], F16, kind="ExternalInput")
    ones128r_d = nc.dram_tensor("ones128r", [1, 128], F16, kind="ExternalInput")
    out_d = nc.dram_tensor("out", [DIM, N], F32, kind="ExternalOutput")

    def ch_cols(t, ch):
        return t[:, ch * CHW:(ch + 1) * CHW]

    with tile.TileContext(nc) as tc:
        with (
            tc.tile_pool(name="wsb", bufs=1) as wsb,
            tc.tile_pool(name="xsb", bufs=1) as xsb,
            tc.tile_pool(name="ysb", bufs=1) as ysb,
            tc.tile_pool(name="big", bufs=1) as big,
            tc.tile_pool(name="small", bufs=2) as small,
            tc.tile_pool(name="ev", bufs=3) as ev,
            tc.tile_pool(name="kv_ps", bufs=2, space="PSUM") as kv_ps,
            tc.tile_pool(name="mm_ps", bufs=2, space="PSUM") as mm_ps,
            tc.tile_pool(name="ktv_ps", bufs=1, space="PSUM") as ktv_ps,
            tc.tile_pool(name="dram", bufs=1, space="DRAM") as dram,
        ):
            def emit_body():
                x0_t = [xsb.tile([128, XL], F16, tag=f"x0{p}", name=f"x0{p}") for p in range(CP)]
                x1_t = [xsb.tile([128, XL], F16, tag=f"x1{p}", name=f"x1{p}") for p in range(CP)]
                wq_t = [wsb.tile([128, INNER], F16, tag=f"wq{p}", name=f"wq{p}") for p in range(CP)]
                wkv_t = [wsb.tile([128, 2 * INNER], F16, tag=f"wkv{p}", name=f"wkv{p}") for p in range(CP)]
                wo_t = [wsb.tile([128, DIM], F16, tag=f"wo{p}", name=f"wo{p}") for p in range(HP)]
                tq_t = [wsb.tile([128, 9], F32, tag=f"tq{p}", name=f"tq{p}") for p in range(CP)]
                bq_t = [wsb.tile([128, 1], F32, tag=f"bq{p}", name=f"bq{p}") for p in range(CP)]
                tk_t = [wsb.tile([128, 9], F32, tag=f"tk{p}", name=f"tk{p}") for p in range(CP)]
                bk_t = [wsb.tile([128, 1], F32, tag=f"bk{p}", name=f"bk{p}") for p in range(CP)]
                bo_t = [wsb.tile([128, 1], F32, tag=f"bo{p}", name=f"bo{p}") for p in range(CP)]
                hsel_t = [wsb.tile([128, HEADS], F16, tag=f"hs{p}", name=f"hs{p}") for p in range(HP)]
                hselT_t = wsb.tile([HEADS, INNER], F16, tag="hsT", name="hsT")
                nones8_t = wsb.tile([HEADS, 1], F16, tag="nones8", name="nones8")
                ones128r_t = wsb.tile([1, 128], F16, tag="ones128r", name="ones128r")

                # kv-conv scalars first (they gate the first ACT tap0 ops),
                # then the first x pieces, then everything else -- each DMA
                # costs ~0.65us of serial issue on the Sync engine
                for p in range(CP):
                    cs = slice(p * 128, (p + 1) * 128)
                    nc.sync.dma_start(out=tk_t[p], in_=tk_d[cs, :])
                    nc.sync.dma_start(out=bk_t[p], in_=bk_d[cs, :])

                G0E = 28 * PC
                G1S = 24 * PC
                for p in range(CP):
                    cs = slice(p * 128, (p + 1) * 128)
                    nc.sync.dma_start(out=x0_t[p][:, 0:G0E], in_=xp_d[cs, 0:G0E])
                    nc.sync.dma_start(out=x1_t[p][:, 0:G0E], in_=xp1_d[cs, 0:G0E])
                for p in range(CP):
                    cs = slice(p * 128, (p + 1) * 128)
                    nc.sync.dma_start(out=tq_t[p], in_=tq_d[cs, :])
                    nc.sync.dma_start(out=bq_t[p], in_=bq_d[cs, :])
                for p in range(CP):
                    cs = slice(p * 128, (p + 1) * 128)
                    nc.sync.dma_start(out=x0_t[p][:, G1S:XL], in_=xp_d[cs, G1S:XL])
                    nc.sync.dma_start(out=x1_t[p][:, G1S:XL], in_=xp1_d[cs, G1S:XL])
                for p in range(CP):
                    cs = slice(p * 128, (p + 1) * 128)
                    nc.sync.dma_start(out=wkv_t[p], in_=wkvT_d[cs, :])
                for p in range(CP):
                    cs = slice(p * 128, (p + 1) * 128)
                    nc.sync.dma_start(out=wq_t[p], in_=wqT_d[cs, :])
                    nc.sync.dma_start(out=bo_t[p], in_=bo_d[cs, :])
                for p in range(HP):
                    cs = slice(p * 128, (p + 1) * 128)
                    nc.sync.dma_start(out=wo_t[p], in_=woT_d[cs, :])
                    nc.sync.dma_start(out=hsel_t[p], in_=hsel_d[cs, :])
                nc.sync.dma_start(out=hselT_t, in_=hselT_d[:, :])
                nc.sync.dma_start(out=nones8_t, in_=nones8_d[:, :])
                nc.sync.dma_start(out=ones128r_t, in_=ones128r_d[:, :])

                # ---- depthwise convs ----
                # tap (0,0) runs on ACT (scale*x + bias, per-partition APs)
                # for every row-group of both convs, hoisted to the front of
                # the ACT queue where the engine is otherwise idle; the
                # remaining 8 taps are DVE mul(4x)+add(2x) pairs.
                RG = ((0, 24), (24, 40), (40, H))

                yq_t = [ysb.tile([128, H, W], F16, tag=f"yq{p}", name=f"yq{p}")
                        for p in range(CP)]
                ykv_t = [ysb.tile([128, H, W], F16, tag=f"ykv{p}", name=f"ykv{p}")
                         for p in range(CP)]

                def conv_tap0(ys, taps, bias):
                    for (r0, r1) in RG:
                        for p in range(CP):
                            # narrow flat slice so the subtile dep tracker
                            # only waits on the x DMA piece actually read
                            xg = x0_t[p][:, r0 * PC:r1 * PC].rearrange(
                                "p (a b) -> p a b", b=PC)
                            nc.scalar.activation(
                                out=ys[p][:, r0:r1, :],
                                in_=xg[:, :, 0:W],
                                func=ACTF.Identity,
                                bias=bias[p], scale=taps[p][:, 0:1])

                conv_tap0(ykv_t, tk_t, bk_t)
                conv_tap0(yq_t, tq_t, bq_t)

                def conv_group(ys, taps, r0, r1):
                    rows = r1 - r0
                    ve = nc.vector
                    for p in range(CP):
                        y = ys[p]
                        for dy in range(3):
                            for dx in range(3):
                                i = dy * 3 + dx
                                if i == 0:
                                    continue
                                off = (r0 + dy) * PC + dx
                                glen = rows * PC
                                t = ysb.tile([128, glen], F16, tag="tconv",
                                             name="tconv", bufs=2,
                                             padded_shape=[128, YL // 2 + PC * 4])
                                if off % 2 == 0:
                                    tsrc = x0_t[p][:, off:off + glen]
                                else:
                                    tsrc = x1_t[p][:, off - 1:off - 1 + glen]
                                ve.tensor_scalar(
                                    out=t, in0=tsrc,
                                    scalar1=taps[p][:, i:i + 1],
                                    scalar2=None, op0=ALU.mult)
                                tv = t.rearrange("p (a b) -> p a b", b=PC)
                                ve.tensor_tensor(
                                    out=y[:, r0:r1, :], in0=tv[:, 0:rows, 0:W],
                                    in1=y[:, r0:r1, :], op=ALU.add)

                # kv-conv groups 1+2 now; group 3 is emitted after q-conv
                # group 1 (kvB chunks 17-24, which read rows >= 40, are also
                # emitted after it -- program order is the semantics)
                conv_group(ykv_t, tk_t, *RG[0])
                conv_group(ykv_t, tk_t, *RG[1])
                y_kv = [y.rearrange("p a b -> p (a b)") for y in ykv_t]
                y_q = [y.rearrange("p a b -> p (a b)") for y in yq_t]

                def y_chunk(y, ch):
                    return y[:, ch * CHW:(ch + 1) * CHW]

                # ---- layout-B k^T,v^T + fused ktv accumulation ----
                # Per 128-position chunk: y-chunk-stationary matmuls (k and
                # v halves share each LDWEIGHTS) into two [128,512] PSUM
                # tiles -> ACT evictions into one fp16 buffer, then 4
                # head-pair matmuls accumulated directly into 4 long-lived
                # single-bank ktv PSUM tiles.  The head-pair matmuls for
                # chunk nb are emitted 2 chunks late so the PE never stalls
                # on the eviction; kA/vA projections for the first conv
                # row-group are interleaved mid-loop to cover the window
                # where the conv hasn't yet produced rows for chunk nb+1.
                ktv_acc = [ktv_ps.tile([128, 128], F32, tag=f"ktva{mp}",
                                       name=f"ktva{mp}") for mp in range(HP)]
                kvchs = {}

                def emit_ktv(nb):
                    c0 = nb * 128
                    m = min(N, c0 + 128) - c0
                    kvch = kvchs.pop(nb)
                    for mp in range(HP):
                        ms = slice(mp * 128, (mp + 1) * 128)
                        vs = slice(INNER + mp * 128, INNER + (mp + 1) * 128)
                        nc.tensor.matmul(out=ktv_acc[mp][:, :],
                                         lhsT=kvch[0:m, ms],
                                         rhs=kvch[0:m, vs],
                                         start=(nb == 0), stop=(nb == NB - 1))

                proj_dst = {}

                def proj_chunks(name, ys, wts, col0, chunks, pool=None):
                    # weight-stationary over chunk pairs: each lhsT slice is
                    # loaded once per pair of 448-col chunks
                    pool = pool or mm_ps
                    if name not in proj_dst:
                        proj_dst[name] = [
                            big.tile([128, N], F16, tag=f"{name}{m}", name=f"{name}{m}")
                            for m in range(HP)]
                    dst = proj_dst[name]
                    for m in range(HP):
                        i = 0
                        while i < len(chunks):
                            pair = chunks[i:i + 2]
                            pss = [pool.tile([128, CHW], F32, tag="kv" if pool is kv_ps else "mm", name="mm")
                                   for _ in pair]
                            for p in range(CP):
                                lhsT = wts[p][:, col0 + m * 128: col0 + (m + 1) * 128]
                                for ps, ch in zip(pss, pair):
                                    nc.tensor.matmul(
                                        out=ps[:, :], lhsT=lhsT,
                                        rhs=y_chunk(ys[p], ch),
                                        start=(p == 0), stop=(p == CP - 1))
                            for ps, ch in zip(pss, pair):
                                nc.scalar.copy(out=ch_cols(dst[m], ch), in_=ps[:, :])
                            i += 2
                    return dst

                def emit_kvB(nbs):
                    for nb in nbs:
                        c0 = nb * 128
                        c1 = min(N, c0 + 128)
                        m = c1 - c0
                        kvch = ev.tile([128, 2 * INNER], F16, tag="kvch", name="kvch")
                        kvchs[nb] = kvch
                        psk = kv_ps.tile([128, INNER], F32, tag="kv", name="kv")
                        psv = kv_ps.tile([128, INNER], F32, tag="kv", name="kv")
                        for p in range(CP):
                            nc.tensor.matmul(
                                out=psk[0:m, :], lhsT=y_kv[p][:, c0:c1],
                                rhs=wkv_t[p][:, 0:INNER],
                                start=(p == 0), stop=(p == CP - 1))
                            nc.tensor.matmul(
                                out=psv[0:m, :], lhsT=y_kv[p][:, c0:c1],
                                rhs=wkv_t[p][:, INNER:2 * INNER],
                                start=(p == 0), stop=(p == CP - 1))
                        nc.scalar.copy(out=kvch[0:m, 0:INNER], in_=psk[0:m, :])
                        nc.scalar.copy(out=kvch[0:m, INNER:2 * INNER], in_=psv[0:m, :])
                        if nb >= 2:
                            emit_ktv(nb - 2)
                        if nb == 10:
                            # conv row-group 0 output (chunks 0-2) keeps the
                            # PE fed while the conv finishes row-group 1;
                            # uses the kv PSUM pool (idle while kvB waits on
                            # the conv) so no WAR cycle with mm-pool users
                            proj_chunks("kA", y_kv, wkv_t, 0, [0, 1, 2],
                                        pool=kv_ps)
                            proj_chunks("vA", y_kv, wkv_t, INNER, [0, 1, 2],
                                        pool=kv_ps)

                # chunks 0-16 only need conv rows < 40 (groups 1+2)
                emit_kvB(range(17))

                # ---- diag machinery (emitted interleaved with q-conv
                # row-groups so qk lands early in the DVE queue) ----
                diag16 = big.tile([HEADS, N], F16, tag="diag16", name="diag16")
                dcols = small.tile([HEADS, NCH], F32, tag="dcols", name="dcols")

                def emit_diag(chunks):
                    kA, qA = proj_dst["kA"], proj_dst["qA"]
                    for ch in chunks:
                        dps = mm_ps.tile([HEADS, CHW], F32, tag="mm", name="dps")
                        qks = []
                        for m in range(HP):
                            qk = ev.tile([128, CHW], F16, tag="qk", name="qk", bufs=4)
                            nc.vector.tensor_tensor(out=qk, in0=ch_cols(kA[m], ch),
                                                    in1=ch_cols(qA[m], ch),
                                                    op=ALU.mult)
                            qks.append(qk)
                        for m in range(HP):
                            nc.tensor.matmul(out=dps[:, :], lhsT=hsel_t[m],
                                             rhs=qks[m],
                                             start=(m == 0), stop=(m == HP - 1))
                        nc.scalar.activation(out=ch_cols(diag16, ch), in_=dps[:, :],
                                             func=ACTF.Identity, bias=0.0, scale=1.0,
                                             accum_out=dcols[:, ch:ch + 1])

                # remaining conv row-groups interleaved with their dependent
                # projections and diag chunks
                kA = proj_chunks("kA", y_kv, wkv_t, 0, [3, 4])
                vA = proj_chunks("vA", y_kv, wkv_t, INNER, [3, 4])
                conv_group(yq_t, tq_t, *RG[0])
                qA = proj_chunks("qA", y_q, wq_t, 0, [0, 1, 2])
                emit_diag([0, 1, 2])
                conv_group(ykv_t, tk_t, *RG[2])
                emit_kvB(range(17, NB))
                emit_ktv(NB - 2)
                emit_ktv(NB - 1)
                kA = proj_chunks("kA", y_kv, wkv_t, 0, [5, 6])
                vA = proj_chunks("vA", y_kv, wkv_t, INNER, [5, 6])
                conv_group(yq_t, tq_t, *RG[1])
                qA = proj_chunks("qA", y_q, wq_t, 0, [3, 4])
                emit_diag([3, 4])
                conv_group(yq_t, tq_t, *RG[2])
                qA = proj_chunks("qA", y_q, wq_t, 0, [5, 6])
                emit_diag([5, 6])

                # block-diagonal [ktv_2m, ktv_2m+1] per m-tile
                ktv_bd = small.tile([128, HP, 128], F16, tag="ktvbd", name="ktvbd")
                nc.vector.memset(ktv_bd, 0.0)
                for mp in range(HP):
                    nc.scalar.copy(out=ktv_bd[0:DIM_HEAD, mp, 0:DIM_HEAD],
                                   in_=ktv_acc[mp][0:DIM_HEAD, 0:DIM_HEAD])
                    nc.scalar.copy(out=ktv_bd[DIM_HEAD:128, mp, DIM_HEAD:128],
                                   in_=ktv_acc[mp][DIM_HEAD:128, DIM_HEAD:128])

                # m0 (negated): dsum = sum(dcols); -m0 = (-1)^T @ dsum;
                # broadcast to [128,1] via a K=1 matmul (no DRAM round trip).
                dsumf = small.tile([HEADS, 1], F32, tag="dsumf", name="dsumf")
                nc.vector.tensor_reduce(out=dsumf, in_=dcols,
                                        axis=mybir.AxisListType.X, op=ALU.add)
                dsum16 = small.tile([HEADS, 1], F16, tag="dsum16", name="dsum16")
                nc.vector.tensor_copy(out=dsum16, in_=dsumf)
                m0_ps = mm_ps.tile([HEADS, CHW], F32, tag="mm", name="m0ps")
                nc.tensor.matmul(out=m0_ps[0:1, 0:1], lhsT=nones8_t, rhs=dsum16,
                                 start=True, stop=True)
                if no_cc:
                    m0r16 = small.tile([1, 1], F16, tag="m0r16", name="m0r16")
                    nc.scalar.copy(out=m0r16, in_=m0_ps[0:1, 0:1])
                else:
                    m0s = small.tile([1, 1], F32, tag="m0s", name="m0s")
                    nc.scalar.copy(out=m0s, in_=m0_ps[0:1, 0:1])
                    cc = dram.tile([1, 1], F32, tag="cc", name="cc")
                    nc.gpsimd.dma_start(out=cc[:, :], in_=m0s)
                    nc.gpsimd.collective_compute(
                        "AllReduce", ALU.add, replica_groups=[list(range(8))],
                        ins=[cc[:, :].opt()], outs=[cc[:, :].opt()])
                    m0r = small.tile([1, 1], F32, tag="m0r", name="m0r")
                    nc.gpsimd.dma_start(out=m0r, in_=cc[:, :])
                    m0r16 = small.tile([1, 1], F16, tag="m0r16", name="m0r16")
                    nc.vector.tensor_copy(out=m0r16, in_=m0r)
                mb_ps = mm_ps.tile([128, CHW], F32, tag="mm", name="mbps")
                nc.tensor.matmul(out=mb_ps[:, 0:1], lhsT=ones128r_t, rhs=m0r16,
                                 start=True, stop=True)
                m0negb = small.tile([128, 1], F32, tag="m0negb", name="m0negb")
                nc.scalar.copy(out=m0negb, in_=mb_ps[:, 0:1])

                # ---- back half, streamed per chunk (stage-major so the
                # per-m PE->ACT->DVE->PE round trips pipeline instead of
                # chaining) ----
                for ch in range(NCH):
                    sb16s = []
                    for m in range(HP):
                        # kv pool banks are free after the kvB phase
                        sb_ps = kv_ps.tile([128, CHW], F32, tag="kv", name="sbps")
                        nc.tensor.matmul(out=sb_ps[:, :],
                                         lhsT=hselT_t[:, m * 128:(m + 1) * 128],
                                         rhs=ch_cols(diag16, ch),
                                         start=True, stop=True)
                        # sb16 = diag - m0 (broadcast), fp16
                        sb16 = ev.tile([128, CHW], F16, tag="sb16", name="sb16",
                                       bufs=4)
                        nc.scalar.activation(out=sb16, in_=sb_ps[:, :],
                                             func=ACTF.Identity,
                                             bias=m0negb, scale=1.0)
                        sb16s.append(sb16)
                    ews = []
                    for m in range(HP):
                        ew = ev.tile([128, CHW], F16, tag="ew", name="ew", bufs=4)
                        nc.vector.tensor_tensor(out=ew, in0=sb16s[m],
                                                in1=ch_cols(vA[m], ch), op=ALU.mult)
                        ews.append(ew)
                    oach = []
                    for m in range(HP):
                        # ktv accumulator banks are free after ktv_bd is built
                        oa_ps = ktv_ps.tile([128, CHW], F32, tag=f"ktva{m}",
                                            name="oaps")
                        nc.tensor.matmul(out=oa_ps[:, :], lhsT=ktv_bd[:, m, :],
                                         rhs=ch_cols(qA[m], ch),
                                         start=True, stop=True)
                        # oa = q@ktv - ew, fused into the DVE eviction
                        oa = ev.tile([128, CHW], F16, tag=f"oa{m}", name=f"oa{m}",
                                     bufs=2)
                        nc.vector.tensor_tensor(out=oa, in0=oa_ps[:, :],
                                                in1=ews[m], op=ALU.subtract)
                        oach.append(oa)
                    for ot in range(CP):
                        ps = mm_ps.tile([128, CHW], F32, tag="mm", name="fps")
                        for kt in range(HP):
                            nc.tensor.matmul(
                                out=ps[:, :],
                                lhsT=wo_t[kt][:, ot * 128:(ot + 1) * 128],
                                rhs=oach[kt],
                                start=(kt == 0), stop=(kt == HP - 1))
                        of = ev.tile([128, CHW], F32, tag="of", name="of", bufs=2)
                        nc.vector.tensor_scalar(out=of, in0=ps[:, :],
                                                scalar1=bo_t[ot], scalar2=None,
                                                op0=ALU.add)
                        nc.sync.dma_start(
                            out=out_d[ot * 128:(ot + 1) * 128,
                                      ch * CHW:(ch + 1) * CHW],
                            in_=of)

            if loop_n is None:
                for _ in range(reps):
                    emit_body()
            else:
                with tc.For_i(0, loop_n, 1):
                    emit_body()
    nc.finalize()
    return nc


def _get_nc(reps: int = 1, loop_n=None, no_cc=False):
    key = (reps, loop_n, no_cc)
    if key not in _CACHE:
        _CACHE[key] = _build(reps, loop_n, no_cc)
    return _CACHE[key]


def prepare_in_maps(inputs):
    """Host-side preprocessing: fold BN, pad/shift x, transpose weights."""
    x = np.asarray(inputs["x"], np.float32)

    def fold(dw, g, b, m, v):
        inv = np.asarray(g, np.float32) / np.sqrt(np.asarray(v, np.float32) + EPS)
        taps = np.asarray(dw, np.float32)[:, 0].reshape(DIM, 9) * inv[:, None]
        bias = np.asarray(b, np.float32) - np.asarray(m, np.float32) * inv
        return (np.ascontiguousarray(taps, np.float32),
                np.ascontiguousarray(bias[:, None], np.float32))

    tq, bq = fold(inputs["wq_dw"], inputs["wq_bn_g"], inputs["wq_bn_b"],
                  inputs["wq_bn_m"], inputs["wq_bn_v"])
    tk, bk = fold(inputs["wkv_dw"], inputs["wkv_bn_g"], inputs["wkv_bn_b"],
                  inputs["wkv_bn_m"], inputs["wkv_bn_v"])
    wqT = _f16((SCALE * np.asarray(inputs["wq_pw"], np.float32)).T)
    wkvT = _f16(np.asarray(inputs["wkv_pw"], np.float32).T)
    woT = _f16(np.asarray(inputs["wo"], np.float32).T)
    bo = np.ascontiguousarray(np.asarray(inputs["bo"], np.float32)[:, None])
    hsel = _f16(np.repeat(np.eye(HEADS, dtype=np.float32), DIM_HEAD, axis=0))
    hselT = _f16(hsel.T)
    nones8 = _f16(-np.ones((HEADS, 1), np.float32))
    ones128r = _f16(np.ones((1, 128), np.float32))

    xpad = np.zeros((B, DIM, PC, PC), np.float16)
    xpad[:, :, 1:1 + H, 1:1 + W] = x.astype(np.float16)
    xflat = np.zeros((B, DIM, XL), np.float16)
    xflat[:, :, :PC * PC] = xpad.reshape(B, DIM, PC * PC)
    xsh = np.zeros_like(xflat)
    xsh[:, :, :XL - 1] = xflat[:, :, 1:]

    shared = dict(tq=tq, bq=bq, tk=tk, bk=bk, wqT=wqT, wkvT=wkvT, woT=woT,
                  bo=bo, hsel=hsel, hselT=hselT, nones8=nones8,
                  ones128r=ones128r)
    return [dict(shared, xp=np.ascontiguousarray(xflat[b]),
                 xp1=np.ascontiguousarray(xsh[b])) for b in range(B)]


def kernel(**inputs) -> np.ndarray:
    from concourse.bass_utils import run_bass_kernel_spmd
    in_maps = prepare_in_maps(inputs)
    nc = _get_nc(1)
    res = run_bass_kernel_spmd(nc, in_maps, list(range(8)))
    out = np.stack([res.results[b]["out"] for b in range(B)])
    return np.ascontiguousarray(out.reshape(B, DIM, H, W).astype(np.float32))
